# revision 1
# baseline (speedup 1.0000x reference)
import sys
sys.path.insert(0, "/opt/trn_rl_repo")
import math
import numpy as np

import concourse.bass as bass
from concourse import bacc, mybir
from concourse.tile import TileContext
from concourse.bass_utils import run_bass_kernel_spmd
from concourse.masks import make_identity

F32 = mybir.dt.float32
F32R = mybir.dt.float32r
I8 = mybir.dt.int8
AF = mybir.ActivationFunctionType
ALU = mybir.AluOpType
AX = mybir.AxisListType

N, G, E = 16384, 32, 524288
D, DFF, ZI, K, L = 512, 1024, 64, 4, 4
UMAP_A, UMAP_B = 1.577, 0.8951
BN_EPS = 1e-5
NCORES = 8
NL = N // NCORES      # 2048 local nodes per core
GL = G // NCORES      # 4 local graphs per core
NG = N // G           # 512 nodes per graph
KB = N // 128         # 128 source blocks

_NC_CACHE = None


def build_nc():
    nc = bacc.Bacc("TRN2", target_bir_lowering=False, debug=False,
                   enable_asserts=True, num_devices=NCORES)

    xt = nc.dram_tensor("xt", (10, NL), F32R, kind="ExternalInput")
    acm = nc.dram_tensor("acm", (N, NL), I8, kind="ExternalInput")
    embw = nc.dram_tensor("embw", (10, D), F32R, kind="ExternalInput")
    gw1 = nc.dram_tensor("gw1", (L * D, D), F32R, kind="ExternalInput")
    gw2 = nc.dram_tensor("gw2", (L * D, D), F32R, kind="ExternalInput")
    mw1 = nc.dram_tensor("mw1", (D, DFF), F32R, kind="ExternalInput")
    mw2 = nc.dram_tensor("mw2", (DFF, DFF), F32R, kind="ExternalInput")
    mw3 = nc.dram_tensor("mw3", (DFF, ZI), F32R, kind="ExternalInput")
    hw1 = nc.dram_tensor("hw1", (K * ZI, ZI), F32R, kind="ExternalInput")
    hw2 = nc.dram_tensor("hw2", (K * ZI, ZI), F32R, kind="ExternalInput")
    gb1_d = nc.dram_tensor("gb1_d", (128, 16), F32, kind="ExternalInput")
    bng_d = nc.dram_tensor("bng_d", (128, 16), F32, kind="ExternalInput")
    bnb_d = nc.dram_tensor("bnb_d", (128, 16), F32, kind="ExternalInput")
    mb1_d = nc.dram_tensor("mb1_d", (128, 8), F32, kind="ExternalInput")
    mb2_d = nc.dram_tensor("mb2_d", (128, 8), F32, kind="ExternalInput")
    mb3_d = nc.dram_tensor("mb3_d", (ZI, 1), F32, kind="ExternalInput")
    hb1_d = nc.dram_tensor("hb1_d", (ZI, K), F32, kind="ExternalInput")
    hb2_d = nc.dram_tensor("hb2_d", (ZI, K), F32, kind="ExternalInput")
    qout = nc.dram_tensor("qout", (GL * K * 4 * 128, NG), F32,
                          kind="ExternalOutput")

    with TileContext(nc) as tc:
        with (
            tc.tile_pool(name="const", bufs=1) as cp,
            tc.tile_pool(name="res", bufs=1) as rp,
            tc.tile_pool(name="ps", bufs=1, space="PSUM") as ps,
            tc.tile_pool(name="dram", bufs=1, space="DRAM") as dp,
        ):
            ident = cp.tile([128, 128], F32, tag="ident")
            make_identity(nc, ident[:])
            nla = cp.tile([128, 1], F32, tag="nla")
            nc.gpsimd.memset(nla[:], -math.log(UMAP_A))
            of = cp.tile([64, 1], F32, tag="of")
            nc.gpsimd.memset(of[:], 1.0)
            onf = cp.tile([1, NG], F32, tag="onf")
            nc.gpsimd.memset(onf[:], 1.0)

            gb1w = cp.tile([128, 16], F32, tag="gb1w")
            nc.sync.dma_start(gb1w[:], gb1_d[:, :])
            bngw = cp.tile([128, 16], F32, tag="bngw")
            nc.sync.dma_start(bngw[:], bng_d[:, :])
            bnbw = cp.tile([128, 16], F32, tag="bnbw")
            nc.sync.dma_start(bnbw[:], bnb_d[:, :])
            mb1w = cp.tile([128, 8], F32, tag="mb1w")
            nc.sync.dma_start(mb1w[:], mb1_d[:, :])
            mb2w = cp.tile([128, 8], F32, tag="mb2w")
            nc.sync.dma_start(mb2w[:], mb2_d[:, :])
            mb3w = cp.tile([ZI, 1], F32, tag="mb3w")
            nc.sync.dma_start(mb3w[:], mb3_d[:, :])
            hb1w = cp.tile([ZI, K], F32, tag="hb1w")
            nc.sync.dma_start(hb1w[:], hb1_d[:, :])
            hb2w = cp.tile([ZI, K], F32, tag="hb2w")
            nc.sync.dma_start(hb2w[:], hb2_d[:, :])

            hT = [rp.tile([128, NL], F32R, tag=f"hT{fc}", name=f"hT{fc}")
                  for fc in range(4)]

            h_slice = [dp.tile([NL, D], F32R, tag=f"hs{i}", name=f"hs{i}")
                       for i in range(4)]
            h_table = [dp.tile([N, D], F32R, tag=f"ht{i}", name=f"ht{i}")
                       for i in range(4)]
            bn_loc = [dp.tile([128, 8], F32, tag=f"bl{i}", name=f"bl{i}")
                      for i in range(L)]
            bn_glob = [dp.tile([128, 8], F32, tag=f"bg{i}", name=f"bg{i}")
                       for i in range(L)]

            # ---------------- embedding ----------------
            with tc.tile_pool(name="emb", bufs=1) as ep:
                xt_sb = ep.tile([10, NL], F32R, tag="xt")
                nc.sync.dma_start(xt_sb[:], xt[:, :])
                ew_sb = ep.tile([10, D], F32R, tag="ew")
                nc.sync.dma_start(ew_sb[:], embw[:, :])
                for i in range(16):
                    p = ps.tile([128, 512], F32, tag=f"b{i % 4}")
                    nc.tensor.matmul(p[:], xt_sb[:, 128 * i:128 * i + 128],
                                     ew_sb[:], start=True, stop=True)
                    hn = ep.tile([128, 512], F32R, tag="hn", bufs=2)
                    nc.vector.tensor_copy(hn[:], p[:])
                    nc.sync.dma_start(h_slice[0][128 * i:128 * i + 128, :],
                                      hn[:])
                for fc in range(4):
                    for j in range(4):
                        p = ps.tile([128, 512], F32, tag=f"b{4 + fc}")
                        nc.tensor.matmul(p[:], ew_sb[:, 128 * fc:128 * fc + 128],
                                         xt_sb[:, 512 * j:512 * j + 512],
                                         start=True, stop=True)
                        nc.vector.tensor_copy(hT[fc][:, 512 * j:512 * j + 512],
                                              p[:])
                nc.gpsimd.collective_compute(
                    "AllGather", ALU.bypass,
                    ins=[h_slice[0][:, :].opt()],
                    outs=[h_table[0][:, :].opt()],
                    replica_groups=[list(range(NCORES))],
                )

            # ---------------- GIN layers ----------------
            with tc.tile_pool(name="gin", bufs=1) as gp:
                for l in range(L):
                    w1s = gp.tile([128, 2048], F32R, tag="w1")
                    w2s = gp.tile([128, 2048], F32R, tag="w2")
                    for ic in range(4):
                        r0 = 512 * l + 128 * ic
                        nc.sync.dma_start(w1s[:, 512 * ic:512 * ic + 512],
                                          gw1[r0:r0 + 128, :])
                        nc.sync.dma_start(w2s[:, 512 * ic:512 * ic + 512],
                                          gw2[r0:r0 + 128, :])
                    mt = [gp.tile([128, NL], F32R, tag=f"mt{fc}", name=f"mt{fc}_{l}")
                          for fc in range(4)]
                    u2 = [gp.tile([128, NL], F32R, tag=f"u2_{fc}", name=f"u2_{fc}_{l}")
                          for fc in range(4)]

                    # aggregation: aggT = h_table.T @ A  (+ hT at evict)
                    for half in range(2):
                        pb = [ps.tile([128, 512], F32, tag=f"b{i}", name=f"pb{i}")
                              for i in range(8)]
                        for k in range(KB):
                            hk_t = gp.tile([128, 512], F32R, tag="hk", bufs=3)
                            nc.sync.dma_start(
                                hk_t[:], h_table[l][128 * k:128 * k + 128, :])
                            ai = gp.tile([128, 1024], I8, tag="ai", bufs=3)
                            nc.sync.dma_start(
                                ai[:], acm[128 * k:128 * k + 128,
                                           1024 * half:1024 * half + 1024])
                            ar = gp.tile([128, 1024], F32R, tag="ar", bufs=3)
                            nc.vector.tensor_copy(ar[:], ai[:])
                            for fc in range(4):
                                for dc in range(2):
                                    nc.tensor.matmul(
                                        pb[fc * 2 + dc][:],
                                        hk_t[:, 128 * fc:128 * fc + 128],
                                        ar[:, 512 * dc:512 * dc + 512],
                                        start=(k == 0), stop=(k == KB - 1))
                        for fc in range(4):
                            for dc in range(2):
                                col = 1024 * half + 512 * dc
                                nc.vector.tensor_tensor(
                                    out=mt[fc][:, col:col + 512],
                                    in0=pb[fc * 2 + dc][:],
                                    in1=hT[fc][:, col:col + 512],
                                    op=ALU.add)

                    # GIN MLP: u1 = relu(m@w1+b1); u2 = u1@w2
                    for j in range(4):
                        ncol = 512 * j
                        u1c = [gp.tile([128, 512], F32R, tag=f"u1_{oc}", bufs=2,
                                        name=f"u1c{oc}") for oc in range(4)]
                        for oc in range(4):
                            p = ps.tile([128, 512], F32, tag=f"b{oc}")
                            for ic in range(4):
                                nc.tensor.matmul(
                                    p[:],
                                    w1s[:, 512 * ic + 128 * oc:
                                        512 * ic + 128 * oc + 128],
                                    mt[ic][:, ncol:ncol + 512],
                                    start=(ic == 0), stop=(ic == 3))
                            nc.scalar.activation(
                                u1c[oc][:], p[:], AF.Relu,
                                bias=gb1w[:, 4 * l + oc:4 * l + oc + 1])
                        for oc in range(4):
                            p = ps.tile([128, 512], F32, tag=f"b{4 + oc}")
                            for ic in range(4):
                                nc.tensor.matmul(
                                    p[:],
                                    w2s[:, 512 * ic + 128 * oc:
                                        512 * ic + 128 * oc + 128],
                                    u1c[ic][:],
                                    start=(ic == 0), stop=(ic == 3))
                            nc.vector.tensor_copy(u2[oc][:, ncol:ncol + 512],
                                                  p[:])

                    # BN stats (local sums) -> AllReduce
                    stat = gp.tile([128, 8], F32, tag="stat")
                    for fc in range(4):
                        nc.vector.reduce_sum(stat[:, fc:fc + 1], u2[fc][:],
                                             axis=AX.X)
                        qacc = gp.tile([128, 1], F32, tag="qacc")
                        for j in range(4):
                            sq = gp.tile([128, 512], F32, tag="sq", bufs=2)
                            nc.scalar.activation(
                                sq[:], u2[fc][:, 512 * j:512 * j + 512],
                                AF.Square)
                            qp = gp.tile([128, 1], F32, tag=f"qp{j}")
                            nc.vector.reduce_sum(qp[:], sq[:], axis=AX.X)
                            if j == 0:
                                nc.vector.tensor_copy(qacc[:], qp[:])
                            else:
                                nc.vector.tensor_tensor(
                                    out=qacc[:], in0=qp[:], in1=qacc[:],
                                    op=ALU.add)
                        nc.vector.tensor_copy(stat[:, 4 + fc:5 + fc], qacc[:])
                    nc.sync.dma_start(bn_loc[l][:, :], stat[:])
                    nc.gpsimd.collective_compute(
                        "AllReduce", ALU.add,
                        ins=[bn_loc[l][:, :].opt()],
                        outs=[bn_glob[l][:, :].opt()],
                        replica_groups=[list(range(NCORES))],
                    )
                    ga = gp.tile([128, 8], F32, tag="ga")
                    nc.sync.dma_start(ga[:], bn_glob[l][:, :])

                    # BN apply + relu + residual (in place into hT)
                    for fc in range(4):
                        mu = gp.tile([128, 1], F32, tag="mu")
                        nc.vector.tensor_scalar(out=mu[:], in0=ga[:, fc:fc + 1],
                                                scalar1=1.0 / N, scalar2=None,
                                                op0=ALU.mult)
                        ex2 = gp.tile([128, 1], F32, tag="ex2")
                        nc.vector.tensor_scalar(out=ex2[:],
                                                in0=ga[:, 4 + fc:5 + fc],
                                                scalar1=1.0 / N, scalar2=None,
                                                op0=ALU.mult)
                        mu2 = gp.tile([128, 1], F32, tag="mu2")
                        nc.vector.tensor_tensor(out=mu2[:], in0=mu[:],
                                                in1=mu[:], op=ALU.mult)
                        var = gp.tile([128, 1], F32, tag="var")
                        nc.vector.tensor_tensor(out=var[:], in0=ex2[:],
                                                in1=mu2[:], op=ALU.subtract)
                        vare = gp.tile([128, 1], F32, tag="vare")
                        nc.vector.tensor_scalar(out=vare[:], in0=var[:],
                                                scalar1=BN_EPS, scalar2=None,
                                                op0=ALU.add)
                        std = gp.tile([128, 1], F32, tag="std")
                        nc.scalar.activation(std[:], vare[:], AF.Sqrt)
                        inv = gp.tile([128, 1], F32, tag="inv")
                        nc.vector.reciprocal(inv[:], std[:])
                        sv = gp.tile([128, 1], F32, tag="sv")
                        nc.vector.tensor_tensor(
                            out=sv[:], in0=inv[:],
                            in1=bngw[:, 4 * l + fc:4 * l + fc + 1],
                            op=ALU.mult)
                        mst = gp.tile([128, 1], F32, tag="mst")
                        nc.vector.tensor_tensor(out=mst[:], in0=mu[:],
                                                in1=sv[:], op=ALU.mult)
                        tv = gp.tile([128, 1], F32, tag="tv")
                        nc.vector.tensor_tensor(
                            out=tv[:], in0=bnbw[:, 4 * l + fc:4 * l + fc + 1],
                            in1=mst[:], op=ALU.subtract)
                        for j in range(4):
                            ncol = 512 * j
                            rt = gp.tile([128, 512], F32R, tag="rt", bufs=2)
                            nc.scalar.activation(
                                rt[:], u2[fc][:, ncol:ncol + 512], AF.Relu,
                                bias=tv[:, 0:1], scale=sv[:, 0:1])
                            nc.vector.tensor_tensor(
                                out=hT[fc][:, ncol:ncol + 512], in0=rt[:],
                                in1=hT[fc][:, ncol:ncol + 512], op=ALU.add)

                    # write updated h back to the replicated table
                    if l < L - 1:
                        for nb in range(16):
                            hn2 = gp.tile([128, 512], F32R, tag="hn2", bufs=2)
                            for fc in range(4):
                                pt = ps.tile([128, 128], F32, tag=f"b{fc}")
                                nc.tensor.transpose(
                                    pt[:],
                                    hT[fc][:, 128 * nb:128 * nb + 128]
                                    .bitcast(F32),
                                    ident[:])
                                nc.vector.tensor_copy(
                                    hn2[:, 128 * fc:128 * fc + 128], pt[:])
                            nc.sync.dma_start(
                                h_slice[l + 1][128 * nb:128 * nb + 128, :],
                                hn2[:])
                        nc.gpsimd.collective_compute(
                            "AllGather", ALU.bypass,
                            ins=[h_slice[l + 1][:, :].opt()],
                            outs=[h_table[l + 1][:, :].opt()],
                            replica_groups=[list(range(NCORES))],
                        )

            # ---------------- final MLP + heads + pairwise ----------------
            with tc.tile_pool(name="fin", bufs=1) as fp:
                mwa = [fp.tile([128, DFF], F32R, tag=f"mw1_{ic}", name=f"mwa{ic}")
                       for ic in range(4)]
                for ic in range(4):
                    nc.sync.dma_start(mwa[ic][:],
                                      mw1[128 * ic:128 * ic + 128, :])
                mwb = [fp.tile([128, DFF], F32R, tag=f"mw2_{ic}", name=f"mwb{ic}")
                       for ic in range(8)]
                for ic in range(8):
                    nc.sync.dma_start(mwb[ic][:],
                                      mw2[128 * ic:128 * ic + 128, :])
                mwc = [fp.tile([128, ZI], F32R, tag=f"mw3_{ic}", name=f"mwc{ic}")
                       for ic in range(8)]
                for ic in range(8):
                    nc.sync.dma_start(mwc[ic][:],
                                      mw3[128 * ic:128 * ic + 128, :])
                hw1s = [fp.tile([ZI, ZI], F32R, tag=f"hw1_{k}", name=f"hw1s{k}")
                        for k in range(K)]
                hw2s = [fp.tile([ZI, ZI], F32R, tag=f"hw2_{k}", name=f"hw2s{k}")
                        for k in range(K)]
                for k in range(K):
                    nc.sync.dma_start(hw1s[k][:], hw1[ZI * k:ZI * k + ZI, :])
                    nc.sync.dma_start(hw2s[k][:], hw2[ZI * k:ZI * k + ZI, :])

                for g in range(GL):
                    gcol = 512 * g
                    z1 = [fp.tile([128, 512], F32R, tag=f"z1_{oc}", name=f"z1_{oc}")
                          for oc in range(8)]
                    for oc in range(8):
                        p = ps.tile([128, 512], F32, tag=f"b{oc}")
                        for ic in range(4):
                            nc.tensor.matmul(
                                p[:],
                                mwa[ic][:, 128 * oc:128 * oc + 128],
                                hT[ic][:, gcol:gcol + 512],
                                start=(ic == 0), stop=(ic == 3))
                        nc.scalar.activation(z1[oc][:], p[:], AF.Relu,
                                             bias=mb1w[:, oc:oc + 1])
                    z2 = [fp.tile([128, 512], F32R, tag=f"z2_{oc}", name=f"z2_{oc}")
                          for oc in range(8)]
                    for oc in range(8):
                        p = ps.tile([128, 512], F32, tag=f"b{oc}")
                        for ic in range(8):
                            nc.tensor.matmul(
                                p[:],
                                mwb[ic][:, 128 * oc:128 * oc + 128],
                                z1[ic][:],
                                start=(ic == 0), stop=(ic == 7))
                        nc.scalar.activation(z2[oc][:], p[:], AF.Relu,
                                             bias=mb2w[:, oc:oc + 1])
                    pz = ps.tile([ZI, 512], F32, tag="b0")
                    for ic in range(8):
                        nc.tensor.matmul(pz[:], mwc[ic][:, 0:ZI], z2[ic][:],
                                         start=(ic == 0), stop=(ic == 7))
                    z3 = fp.tile([ZI, 512], F32R, tag="z3")
                    nc.vector.tensor_tensor(
                        out=z3[:], in0=pz[:],
                        in1=mb3w[:, 0:1].to_broadcast([ZI, 512])[:],
                        op=ALU.add)
                    for k in range(K):
                        p1 = ps.tile([ZI, 512], F32, tag="b1")
                        nc.tensor.matmul(p1[:], hw1s[k][:], z3[:],
                                         start=True, stop=True)
                        h1 = fp.tile([ZI, 512], F32R, tag="h1", bufs=2)
                        nc.scalar.activation(h1[:], p1[:], AF.Relu,
                                             bias=hb1w[:, k:k + 1])
                        p2 = ps.tile([ZI, 512], F32, tag="b2")
                        nc.tensor.matmul(p2[:], hw2s[k][:], h1[:],
                                         start=True, stop=True)
                        hkt = fp.tile([ZI, 512], F32, tag="hkt", bufs=2)
                        nc.vector.tensor_tensor(
                            out=hkt[:], in0=p2[:],
                            in1=hb2w[:, k:k + 1].to_broadcast([ZI, 512])[:],
                            op=ALU.add)
                        hm2 = fp.tile([ZI, 512], F32, tag="hm2", bufs=2)
                        nc.vector.tensor_scalar(out=hm2[:], in0=hkt[:],
                                                scalar1=-2.0, scalar2=None,
                                                op0=ALU.mult)
                        sqt = fp.tile([ZI, 512], F32, tag="sqt", bufs=2)
                        nc.vector.tensor_tensor(out=sqt[:], in0=hkt[:],
                                                in1=hkt[:], op=ALU.mult)
                        pr = ps.tile([1, 512], F32, tag="b3")
                        nc.tensor.matmul(pr[:], of[:], sqt[:],
                                         start=True, stop=True)
                        rsb = fp.tile([1, 512], F32, tag="rsb", bufs=2)
                        nc.vector.tensor_copy(rsb[:], pr[:])
                        for mb in range(4):
                            pd = ps.tile([128, 512], F32, tag=f"b{4 + mb}")
                            nc.tensor.matmul(pd[:],
                                             hm2[:, 128 * mb:128 * mb + 128],
                                             hkt[:], start=True, stop=False)
                            nc.tensor.matmul(pd[:], onf[:, 0:128], rsb[:],
                                             start=False, stop=False,
                                             skip_group_check=True)
                            nc.tensor.matmul(pd[:],
                                             rsb[:, 128 * mb:128 * mb + 128],
                                             onf[:], start=False, stop=True,
                                             skip_group_check=True)
                            d2t = fp.tile([128, 512], F32, tag="d2", bufs=2)
                            nc.vector.tensor_scalar(out=d2t[:], in0=pd[:],
                                                    scalar1=1e-12,
                                                    scalar2=None, op0=ALU.max)
                            lnt = fp.tile([128, 512], F32, tag="ln", bufs=2)
                            nc.scalar.activation(lnt[:], d2t[:], AF.Ln)
                            qt = fp.tile([128, 512], F32, tag="qt", bufs=3)
                            nc.scalar.activation(qt[:], lnt[:], AF.Sigmoid,
                                                 bias=nla[:, 0:1],
                                                 scale=-UMAP_B)
                            row = ((g * K + k) * 4 + mb) * 128
                            nc.sync.dma_start(qout[row:row + 128, :], qt[:])
    nc.compile()
    return nc


def _host_prep(inputs):
    x = np.asarray(inputs["x"], np.float32)
    edge_index = np.asarray(inputs["edge_index"], np.int64)
    src, dst = edge_index[0], edge_index[1]

    shared = {
        "embw": np.ascontiguousarray(np.vstack(
            [np.asarray(inputs["emb_w"], np.float32),
             np.asarray(inputs["emb_b"], np.float32)[None, :]])),
        "gw1": np.ascontiguousarray(
            np.asarray(inputs["gin_w1"], np.float32).reshape(L * D, D)),
        "gw2": np.ascontiguousarray(
            np.asarray(inputs["gin_w2"], np.float32).reshape(L * D, D)),
        "mw1": np.ascontiguousarray(np.asarray(inputs["mlp_w1"], np.float32)),
        "mw2": np.ascontiguousarray(np.asarray(inputs["mlp_w2"], np.float32)),
        "mw3": np.ascontiguousarray(np.asarray(inputs["mlp_w3"], np.float32)),
        "hw1": np.ascontiguousarray(
            np.asarray(inputs["head_w1"], np.float32).reshape(K * ZI, ZI)),
        "hw2": np.ascontiguousarray(
            np.asarray(inputs["head_w2"], np.float32).reshape(K * ZI, ZI)),
        "gb1_d": np.ascontiguousarray(
            np.asarray(inputs["gin_b1"], np.float32)
            .reshape(L, 4, 128).transpose(2, 0, 1).reshape(128, 16)),
        "bng_d": np.ascontiguousarray(
            np.asarray(inputs["bn_g"], np.float32)
            .reshape(L, 4, 128).transpose(2, 0, 1).reshape(128, 16)),
        "bnb_d": np.ascontiguousarray(
            np.asarray(inputs["bn_b"], np.float32)
            .reshape(L, 4, 128).transpose(2, 0, 1).reshape(128, 16)),
        "mb1_d": np.ascontiguousarray(
            np.asarray(inputs["mlp_b1"], np.float32).reshape(8, 128).T),
        "mb2_d": np.ascontiguousarray(
            np.asarray(inputs["mlp_b2"], np.float32).reshape(8, 128).T),
        "mb3_d": np.ascontiguousarray(
            np.asarray(inputs["mlp_b3"], np.float32)[:, None]),
        "hb1_d": np.ascontiguousarray(
            np.asarray(inputs["head_b1"], np.float32).T),
        "hb2_d": np.ascontiguousarray(
            np.asarray(inputs["head_b2"], np.float32).T),
    }

    in_maps = []
    ones_row = np.ones((1, NL), np.float32)
    for c in range(NCORES):
        lo = NL * c
        mask = (dst >= lo) & (dst < lo + NL)
        flat = src[mask] * NL + (dst[mask] - lo)
        a = np.bincount(flat, minlength=N * NL).astype(np.int8)
        m = dict(shared)
        m["acm"] = np.ascontiguousarray(a.reshape(N, NL))
        m["xt"] = np.ascontiguousarray(
            np.vstack([x[lo:lo + NL].T, ones_row]))
        in_maps.append(m)
    return in_maps


def kernel(**inputs) -> np.ndarray:
    global _NC_CACHE
    if _NC_CACHE is None:
        _NC_CACHE = build_nc()
    nc = _NC_CACHE
    in_maps = _host_prep(inputs)
    res = run_bass_kernel_spmd(nc, in_maps, core_ids=list(range(NCORES)))
    out = np.concatenate(
        [np.asarray(res.results[c]["qout"]).reshape(GL, K, NG, NG)
         for c in range(NCORES)], axis=0)
    return out



# revision 10
# speedup vs baseline: 1.2641x; 1.2641x over previous
import sys
sys.path.insert(0, "/opt/trn_rl_repo")
import math
import numpy as np
import ml_dtypes

import concourse.bass as bass
from concourse import bacc, mybir
from concourse.tile import TileContext
from concourse.bass_utils import run_bass_kernel_spmd
from concourse.masks import make_identity

F32 = mybir.dt.float32
F32R = mybir.dt.float32r
F8 = mybir.dt.float8e4
AF = mybir.ActivationFunctionType
ALU = mybir.AluOpType
AX = mybir.AxisListType
DR = mybir.MatmulPerfMode.DoubleRow

N, G, E = 16384, 32, 524288
D, DFF, ZI, K, L = 512, 1024, 64, 4, 4
UMAP_A, UMAP_B = 1.577, 0.8951
BN_EPS = 1e-5
NCORES = 8
NL = N // NCORES      # 2048 local nodes per core
GL = G // NCORES      # 4 local graphs per core
NG = N // G           # 512 nodes per graph
KB2 = N // 256        # 64 paired source blocks (256 src nodes each)

_NC_CACHE = None


def build_nc():
    nc = bacc.Bacc("TRN2", target_bir_lowering=False, debug=False,
                   enable_asserts=True, num_devices=NCORES)

    xt = nc.dram_tensor("xt", (10, NL), F32R, kind="ExternalInput")
    acm8 = nc.dram_tensor("acm8", (128, KB2, 2, NL), F8, kind="ExternalInput")
    embw = nc.dram_tensor("embw", (10, D), F32R, kind="ExternalInput")
    gw1 = nc.dram_tensor("gw1", (L * D, D), F32R, kind="ExternalInput")
    gw2 = nc.dram_tensor("gw2", (L * D, D), F32R, kind="ExternalInput")
    mw1 = nc.dram_tensor("mw1", (D, DFF), F32R, kind="ExternalInput")
    mw2 = nc.dram_tensor("mw2", (DFF, DFF), F32R, kind="ExternalInput")
    mw3 = nc.dram_tensor("mw3", (DFF, ZI), F32R, kind="ExternalInput")
    hw1 = nc.dram_tensor("hw1", (K * ZI, ZI), F32R, kind="ExternalInput")
    hw2 = nc.dram_tensor("hw2", (K * ZI, ZI), F32R, kind="ExternalInput")
    gb1_d = nc.dram_tensor("gb1_d", (128, 16), F32, kind="ExternalInput")
    bng_d = nc.dram_tensor("bng_d", (128, 16), F32, kind="ExternalInput")
    bnb_d = nc.dram_tensor("bnb_d", (128, 16), F32, kind="ExternalInput")
    mb1_d = nc.dram_tensor("mb1_d", (128, 8), F32, kind="ExternalInput")
    mb2_d = nc.dram_tensor("mb2_d", (128, 8), F32, kind="ExternalInput")
    mb3_d = nc.dram_tensor("mb3_d", (ZI, 1), F32, kind="ExternalInput")
    hb1_d = nc.dram_tensor("hb1_d", (ZI, K), F32, kind="ExternalInput")
    hb2_d = nc.dram_tensor("hb2_d", (ZI, K), F32, kind="ExternalInput")
    qout = nc.dram_tensor("qout", (GL * K * 4 * 128, NG), F32,
                          kind="ExternalOutput")

    with TileContext(nc) as tc:
        with (
            tc.tile_pool(name="const", bufs=1) as cp,
            tc.tile_pool(name="res", bufs=1) as rp,
            tc.tile_pool(name="ps", bufs=1, space="PSUM") as ps,
            tc.tile_pool(name="dram", bufs=1, space="DRAM") as dp,
        ):
            ident = cp.tile([128, 128], F32, tag="ident")
            make_identity(nc, ident[:])
            nla = cp.tile([128, 1], F32, tag="nla")
            nc.gpsimd.memset(nla[:], -math.log(UMAP_A))
            of = cp.tile([64, 1], F32, tag="of")
            nc.gpsimd.memset(of[:], 1.0)
            onf = cp.tile([1, NG], F32, tag="onf")
            nc.gpsimd.memset(onf[:], 1.0)

            gb1w = cp.tile([128, 16], F32, tag="gb1w")
            nc.sync.dma_start(gb1w[:], gb1_d[:, :])
            bngw = cp.tile([128, 16], F32, tag="bngw")
            nc.sync.dma_start(bngw[:], bng_d[:, :])
            bnbw = cp.tile([128, 16], F32, tag="bnbw")
            nc.sync.dma_start(bnbw[:], bnb_d[:, :])
            mb1w = cp.tile([128, 8], F32, tag="mb1w")
            nc.sync.dma_start(mb1w[:], mb1_d[:, :])
            mb2w = cp.tile([128, 8], F32, tag="mb2w")
            nc.sync.dma_start(mb2w[:], mb2_d[:, :])
            mb3w = cp.tile([ZI, 1], F32, tag="mb3w")
            nc.sync.dma_start(mb3w[:], mb3_d[:, :])
            hb1w = cp.tile([ZI, K], F32, tag="hb1w")
            nc.sync.dma_start(hb1w[:], hb1_d[:, :])
            hb2w = cp.tile([ZI, K], F32, tag="hb2w")
            nc.sync.dma_start(hb2w[:], hb2_d[:, :])

            hT = [rp.tile([128, NL], F32R, tag=f"hT{fc}", name=f"hT{fc}")
                  for fc in range(4)]

# layout: [kb2][p][two][hilo*512+d] — hi and lo fp8 halves of h
            slice8 = [dp.tile([NL // 256, 128, 2, 2 * D], F8, tag=f"s8_{i}",
                              name=f"s8_{i}") for i in range(L)]
            table8 = [dp.tile([KB2, 128, 2, 2 * D], F8, tag=f"t8_{i}",
                              name=f"t8_{i}", addr_space="Shared")
                      for i in range(L)]
            bn_loc = [dp.tile([128, 8], F32, tag=f"bl{i}", name=f"bl{i}")
                      for i in range(L)]
            bn_glob = [dp.tile([128, 8], F32, tag=f"bg{i}", name=f"bg{i}",
                               addr_space="Shared")
                       for i in range(L)]

            # ---------------- embedding ----------------
            with tc.tile_pool(name="emb", bufs=1) as ep:
                xt_sb = ep.tile([10, NL], F32R, tag="xt")
                nc.sync.dma_start(xt_sb[:], xt[:, :])
                ew_sb = ep.tile([10, D], F32R, tag="ew")
                nc.sync.dma_start(ew_sb[:], embw[:, :])
                for i in range(16):
                    p = ps.tile([128, 512], F32, tag=f"b{i % 4}")
                    nc.tensor.matmul(p[:], xt_sb[:, 128 * i:128 * i + 128],
                                     ew_sb[:], start=True, stop=True)
                    hn8 = ep.tile([128, 2 * 512], F8, tag="hn8", bufs=2)
                    nc.vector.tensor_copy(hn8[:, 0:512], p[:])
                    h32 = ep.tile([128, 512], F32, tag="h32", bufs=2)
                    nc.vector.tensor_copy(h32[:], hn8[:, 0:512])
                    nc.vector.tensor_tensor(out=hn8[:, 512:1024], in0=p[:],
                                            in1=h32[:], op=ALU.subtract)
                    nc.sync.dma_start(slice8[0][i // 2, :, i % 2, :], hn8[:])
                for fc in range(4):
                    for j in range(4):
                        p = ps.tile([128, 512], F32, tag=f"b{4 + fc}")
                        nc.tensor.matmul(p[:], ew_sb[:, 128 * fc:128 * fc + 128],
                                         xt_sb[:, 512 * j:512 * j + 512],
                                         start=True, stop=True)
                        nc.vector.tensor_copy(hT[fc][:, 512 * j:512 * j + 512],
                                              p[:])
                nc.gpsimd.collective_compute(
                    "AllGather", ALU.bypass,
                    ins=[slice8[0][:, :, :, :].opt()],
                    outs=[table8[0][:, :, :, :].opt()],
                    replica_groups=[list(range(NCORES))],
                )

            # ---------------- GIN layers ----------------
            with tc.tile_pool(name="gin", bufs=1) as gp:
                for l in range(L):
                    w1s = gp.tile([128, 2048], F32R, tag="w1")
                    w2s = gp.tile([128, 2048], F32R, tag="w2")
                    for ic in range(4):
                        r0 = 512 * l + 128 * ic
                        nc.sync.dma_start(w1s[:, 512 * ic:512 * ic + 512],
                                          gw1[r0:r0 + 128, :])
                        nc.sync.dma_start(w2s[:, 512 * ic:512 * ic + 512],
                                          gw2[r0:r0 + 128, :])

                    mt = [gp.tile([128, NL], F32R, tag=f"mt{fc}",
                                  name=f"mt{fc}_{l}") for fc in range(4)]

                    # aggregation: aggT = (hi + lo).T @ A8 (DoubleRow fp8) + hT
                    for half in range(2):
                        pb = [ps.tile([128, 512], F32, tag=f"b{i}",
                                      name=f"pb{i}") for i in range(8)]
                        for kb2 in range(KB2):
                            a8 = gp.tile([128, 2, 1024], F8, tag="a8", bufs=4)
                            nc.sync.dma_start(
                                a8[:],
                                acm8[:, kb2, :,
                                     1024 * half:1024 * half + 1024])
                            hk8 = gp.tile([128, 2, 1024], F8, tag="hk8",
                                          bufs=4)
                            nc.sync.dma_start(hk8[:], table8[l][kb2, :, :, :])
                            for d in range(4):
                                for hilo in range(2):
                                    lhs = hk8[:, :,
                                              512 * hilo + 128 * d:
                                              512 * hilo + 128 * d + 128]
                                    for dc in range(2):
                                        nc.tensor.matmul(
                                            pb[2 * d + dc][:], lhs,
                                            a8[:, :, 512 * dc:512 * dc + 512],
                                            start=(kb2 == 0 and hilo == 0),
                                            stop=(kb2 == KB2 - 1 and hilo == 1),
                                            perf_mode=DR)
                        for d in range(4):
                            for dc in range(2):
                                col = 1024 * half + 512 * dc
                                nc.vector.tensor_tensor(
                                    out=mt[d][:, col:col + 512],
                                    in0=pb[2 * d + dc][:],
                                    in1=hT[d][:, col:col + 512],
                                    op=ALU.add)

                    # GIN MLP: u1 = relu(m@w1+b1); u2 = u1@w2 (into mt)
                    for j in range(4):
                        ncol = 512 * j
                        u1c = [gp.tile([128, 512], F32R, tag=f"u1_{oc}", bufs=2,
                                       name=f"u1c{oc}") for oc in range(4)]
                        for oc in range(4):
                            p = ps.tile([128, 512], F32, tag=f"b{oc}")
                            for ic in range(4):
                                nc.tensor.matmul(
                                    p[:],
                                    w1s[:, 512 * ic + 128 * oc:
                                        512 * ic + 128 * oc + 128],
                                    mt[ic][:, ncol:ncol + 512],
                                    start=(ic == 0), stop=(ic == 3))
                            nc.scalar.activation(
                                u1c[oc][:], p[:], AF.Relu,
                                bias=gb1w[:, 4 * l + oc:4 * l + oc + 1])
                        for oc in range(4):
                            p = ps.tile([128, 512], F32, tag=f"b{4 + oc}")
                            for ic in range(4):
                                nc.tensor.matmul(
                                    p[:],
                                    w2s[:, 512 * ic + 128 * oc:
                                        512 * ic + 128 * oc + 128],
                                    u1c[ic][:],
                                    start=(ic == 0), stop=(ic == 3))
                            nc.vector.tensor_copy(mt[oc][:, ncol:ncol + 512],
                                                  p[:])

                    # BN stats (local sums) -> AllReduce
                    stat = gp.tile([128, 8], F32, tag="stat")
                    for fc in range(4):
                        nc.vector.reduce_sum(stat[:, fc:fc + 1], mt[fc][:],
                                             axis=AX.X)
                        qacc = gp.tile([128, 1], F32, tag="qacc")
                        for j in range(4):
                            sq = gp.tile([128, 512], F32, tag="sq", bufs=2)
                            nc.scalar.activation(
                                sq[:], mt[fc][:, 512 * j:512 * j + 512],
                                AF.Square)
                            qp = gp.tile([128, 1], F32, tag=f"qp{j}")
                            nc.vector.reduce_sum(qp[:], sq[:], axis=AX.X)
                            if j == 0:
                                nc.vector.tensor_copy(qacc[:], qp[:])
                            else:
                                nc.vector.tensor_tensor(
                                    out=qacc[:], in0=qp[:], in1=qacc[:],
                                    op=ALU.add)
                        nc.vector.tensor_copy(stat[:, 4 + fc:5 + fc], qacc[:])
                    nc.sync.dma_start(bn_loc[l][:, :], stat[:])
                    nc.gpsimd.collective_compute(
                        "AllReduce", ALU.add,
                        ins=[bn_loc[l][:, :].opt()],
                        outs=[bn_glob[l][:, :].opt()],
                        replica_groups=[list(range(NCORES))],
                    )
                    ga = gp.tile([128, 8], F32, tag="ga")
                    nc.sync.dma_start(ga[:], bn_glob[l][:, :])

                    # BN apply + relu + residual (in place into hT)
                    for fc in range(4):
                        mu = gp.tile([128, 1], F32, tag="mu")
                        nc.vector.tensor_scalar(out=mu[:], in0=ga[:, fc:fc + 1],
                                                scalar1=1.0 / N, scalar2=None,
                                                op0=ALU.mult)
                        ex2 = gp.tile([128, 1], F32, tag="ex2")
                        nc.vector.tensor_scalar(out=ex2[:],
                                                in0=ga[:, 4 + fc:5 + fc],
                                                scalar1=1.0 / N, scalar2=None,
                                                op0=ALU.mult)
                        mu2 = gp.tile([128, 1], F32, tag="mu2")
                        nc.vector.tensor_tensor(out=mu2[:], in0=mu[:],
                                                in1=mu[:], op=ALU.mult)
                        var = gp.tile([128, 1], F32, tag="var")
                        nc.vector.tensor_tensor(out=var[:], in0=ex2[:],
                                                in1=mu2[:], op=ALU.subtract)
                        vare = gp.tile([128, 1], F32, tag="vare")
                        nc.vector.tensor_scalar(out=vare[:], in0=var[:],
                                                scalar1=BN_EPS, scalar2=None,
                                                op0=ALU.add)
                        std = gp.tile([128, 1], F32, tag="std")
                        nc.scalar.activation(std[:], vare[:], AF.Sqrt)
                        inv = gp.tile([128, 1], F32, tag="inv")
                        nc.vector.reciprocal(inv[:], std[:])
                        sv = gp.tile([128, 1], F32, tag="sv")
                        nc.vector.tensor_tensor(
                            out=sv[:], in0=inv[:],
                            in1=bngw[:, 4 * l + fc:4 * l + fc + 1],
                            op=ALU.mult)
                        mst = gp.tile([128, 1], F32, tag="mst")
                        nc.vector.tensor_tensor(out=mst[:], in0=mu[:],
                                                in1=sv[:], op=ALU.mult)
                        tv = gp.tile([128, 1], F32, tag="tv")
                        nc.vector.tensor_tensor(
                            out=tv[:], in0=bnbw[:, 4 * l + fc:4 * l + fc + 1],
                            in1=mst[:], op=ALU.subtract)
                        for j in range(4):
                            ncol = 512 * j
                            rt = gp.tile([128, 512], F32R, tag="rt", bufs=2)
                            nc.scalar.activation(
                                rt[:], mt[fc][:, ncol:ncol + 512], AF.Relu,
                                bias=tv[:, 0:1], scale=sv[:, 0:1])
                            nc.vector.tensor_tensor(
                                out=hT[fc][:, ncol:ncol + 512], in0=rt[:],
                                in1=hT[fc][:, ncol:ncol + 512], op=ALU.add)

                    # write updated h (fp8) back to the replicated table
                    if l < L - 1:
                        for nb in range(16):
                            hn2 = gp.tile([128, 2 * 512], F8, tag="hn2",
                                          bufs=2)
                            for fc in range(4):
                                pt = ps.tile([128, 128], F32, tag=f"b{fc}")
                                nc.tensor.transpose(
                                    pt[:],
                                    hT[fc][:, 128 * nb:128 * nb + 128]
                                    .bitcast(F32),
                                    ident[:])
                                nc.vector.tensor_copy(
                                    hn2[:, 128 * fc:128 * fc + 128], pt[:])
                                h32 = gp.tile([128, 128], F32, tag="h32",
                                              bufs=2)
                                nc.vector.tensor_copy(
                                    h32[:], hn2[:, 128 * fc:128 * fc + 128])
                                nc.vector.tensor_tensor(
                                    out=hn2[:, 512 + 128 * fc:
                                            512 + 128 * fc + 128],
                                    in0=pt[:], in1=h32[:], op=ALU.subtract)
                            nc.sync.dma_start(
                                slice8[l + 1][nb // 2, :, nb % 2, :], hn2[:])
                        nc.gpsimd.collective_compute(
                            "AllGather", ALU.bypass,
                            ins=[slice8[l + 1][:, :, :, :].opt()],
                            outs=[table8[l + 1][:, :, :, :].opt()],
                            replica_groups=[list(range(NCORES))],
                        )

            # ---------------- final MLP + heads + pairwise ----------------
            with tc.tile_pool(name="fin", bufs=1) as fp:
                mwa = [fp.tile([128, DFF], F32R, tag=f"mw1_{ic}", name=f"mwa{ic}")
                       for ic in range(4)]
                for ic in range(4):
                    nc.sync.dma_start(mwa[ic][:],
                                      mw1[128 * ic:128 * ic + 128, :])
                mwb = [fp.tile([128, DFF], F32R, tag=f"mw2_{ic}", name=f"mwb{ic}")
                       for ic in range(8)]
                for ic in range(8):
                    nc.sync.dma_start(mwb[ic][:],
                                      mw2[128 * ic:128 * ic + 128, :])
                mwc = [fp.tile([128, ZI], F32R, tag=f"mw3_{ic}", name=f"mwc{ic}")
                       for ic in range(8)]
                for ic in range(8):
                    nc.sync.dma_start(mwc[ic][:],
                                      mw3[128 * ic:128 * ic + 128, :])
                hw1s = [fp.tile([ZI, ZI], F32R, tag=f"hw1_{k}", name=f"hw1s{k}")
                        for k in range(K)]
                hw2s = [fp.tile([ZI, ZI], F32R, tag=f"hw2_{k}", name=f"hw2s{k}")
                        for k in range(K)]
                for k in range(K):
                    nc.sync.dma_start(hw1s[k][:], hw1[ZI * k:ZI * k + ZI, :])
                    nc.sync.dma_start(hw2s[k][:], hw2[ZI * k:ZI * k + ZI, :])

                for g in range(GL):
                    gcol = 512 * g
                    z1 = [fp.tile([128, 512], F32R, tag=f"z1_{oc}", name=f"z1_{oc}")
                          for oc in range(8)]
                    for oc in range(8):
                        p = ps.tile([128, 512], F32, tag=f"b{oc}")
                        for ic in range(4):
                            nc.tensor.matmul(
                                p[:],
                                mwa[ic][:, 128 * oc:128 * oc + 128],
                                hT[ic][:, gcol:gcol + 512],
                                start=(ic == 0), stop=(ic == 3))
                        nc.scalar.activation(z1[oc][:], p[:], AF.Relu,
                                             bias=mb1w[:, oc:oc + 1])
                    z2 = [fp.tile([128, 512], F32R, tag=f"z2_{oc}", name=f"z2_{oc}")
                          for oc in range(8)]
                    for oc in range(8):
                        p = ps.tile([128, 512], F32, tag=f"b{oc}")
                        for ic in range(8):
                            nc.tensor.matmul(
                                p[:],
                                mwb[ic][:, 128 * oc:128 * oc + 128],
                                z1[ic][:],
                                start=(ic == 0), stop=(ic == 7))
                        nc.scalar.activation(z2[oc][:], p[:], AF.Relu,
                                             bias=mb2w[:, oc:oc + 1])
                    pz = ps.tile([ZI, 512], F32, tag="b0")
                    for ic in range(8):
                        nc.tensor.matmul(pz[:], mwc[ic][:, 0:ZI], z2[ic][:],
                                         start=(ic == 0), stop=(ic == 7))
                    z3 = fp.tile([ZI, 512], F32R, tag="z3")
                    nc.vector.tensor_tensor(
                        out=z3[:], in0=pz[:],
                        in1=mb3w[:, 0:1].to_broadcast([ZI, 512])[:],
                        op=ALU.add)
                    for k in range(K):
                        p1 = ps.tile([ZI, 512], F32, tag="b1")
                        nc.tensor.matmul(p1[:], hw1s[k][:], z3[:],
                                         start=True, stop=True)
                        h1 = fp.tile([ZI, 512], F32R, tag="h1", bufs=2)
                        nc.scalar.activation(h1[:], p1[:], AF.Relu,
                                             bias=hb1w[:, k:k + 1])
                        p2 = ps.tile([ZI, 512], F32, tag="b2")
                        nc.tensor.matmul(p2[:], hw2s[k][:], h1[:],
                                         start=True, stop=True)
                        hkt = fp.tile([ZI, 512], F32, tag="hkt", bufs=2)
                        nc.vector.tensor_tensor(
                            out=hkt[:], in0=p2[:],
                            in1=hb2w[:, k:k + 1].to_broadcast([ZI, 512])[:],
                            op=ALU.add)
                        hm2 = fp.tile([ZI, 512], F32, tag="hm2", bufs=2)
                        nc.vector.tensor_scalar(out=hm2[:], in0=hkt[:],
                                                scalar1=-2.0, scalar2=None,
                                                op0=ALU.mult)
                        sqt = fp.tile([ZI, 512], F32, tag="sqt", bufs=2)
                        nc.vector.tensor_tensor(out=sqt[:], in0=hkt[:],
                                                in1=hkt[:], op=ALU.mult)
                        pr = ps.tile([1, 512], F32, tag="b3")
                        nc.tensor.matmul(pr[:], of[:], sqt[:],
                                         start=True, stop=True)
                        rsb = fp.tile([1, 512], F32, tag="rsb", bufs=2)
                        nc.vector.tensor_copy(rsb[:], pr[:])
                        for mb in range(4):
                            pd = ps.tile([128, 512], F32, tag=f"b{4 + mb}")
                            nc.tensor.matmul(pd[:],
                                             hm2[:, 128 * mb:128 * mb + 128],
                                             hkt[:], start=True, stop=False)
                            nc.tensor.matmul(pd[:], onf[:, 0:128], rsb[:],
                                             start=False, stop=False,
                                             skip_group_check=True)
                            nc.tensor.matmul(pd[:],
                                             rsb[:, 128 * mb:128 * mb + 128],
                                             onf[:], start=False, stop=True,
                                             skip_group_check=True)
                            d2t = fp.tile([128, 512], F32, tag="d2", bufs=2)
                            nc.vector.tensor_scalar(out=d2t[:], in0=pd[:],
                                                    scalar1=1e-12,
                                                    scalar2=None, op0=ALU.max)
                            lnt = fp.tile([128, 512], F32, tag="ln", bufs=2)
                            nc.scalar.activation(lnt[:], d2t[:], AF.Ln)
                            qt = fp.tile([128, 512], F32, tag="qt", bufs=3)
                            nc.scalar.activation(qt[:], lnt[:], AF.Sigmoid,
                                                 bias=nla[:, 0:1],
                                                 scale=-UMAP_B)
                            row = ((g * K + k) * 4 + mb) * 128
                            nc.sync.dma_start(qout[row:row + 128, :], qt[:])
    nc.compile()
    return nc


def _host_prep(inputs):
    x = np.asarray(inputs["x"], np.float32)
    edge_index = np.asarray(inputs["edge_index"], np.int64)
    src, dst = edge_index[0], edge_index[1]

    shared = {
        "embw": np.ascontiguousarray(np.vstack(
            [np.asarray(inputs["emb_w"], np.float32),
             np.asarray(inputs["emb_b"], np.float32)[None, :]])),
        "gw1": np.ascontiguousarray(
            np.asarray(inputs["gin_w1"], np.float32).reshape(L * D, D)),
        "gw2": np.ascontiguousarray(
            np.asarray(inputs["gin_w2"], np.float32).reshape(L * D, D)),
        "mw1": np.ascontiguousarray(np.asarray(inputs["mlp_w1"], np.float32)),
        "mw2": np.ascontiguousarray(np.asarray(inputs["mlp_w2"], np.float32)),
        "mw3": np.ascontiguousarray(np.asarray(inputs["mlp_w3"], np.float32)),
        "hw1": np.ascontiguousarray(
            np.asarray(inputs["head_w1"], np.float32).reshape(K * ZI, ZI)),
        "hw2": np.ascontiguousarray(
            np.asarray(inputs["head_w2"], np.float32).reshape(K * ZI, ZI)),
        "gb1_d": np.ascontiguousarray(
            np.asarray(inputs["gin_b1"], np.float32)
            .reshape(L, 4, 128).transpose(2, 0, 1).reshape(128, 16)),
        "bng_d": np.ascontiguousarray(
            np.asarray(inputs["bn_g"], np.float32)
            .reshape(L, 4, 128).transpose(2, 0, 1).reshape(128, 16)),
        "bnb_d": np.ascontiguousarray(
            np.asarray(inputs["bn_b"], np.float32)
            .reshape(L, 4, 128).transpose(2, 0, 1).reshape(128, 16)),
        "mb1_d": np.ascontiguousarray(
            np.asarray(inputs["mlp_b1"], np.float32).reshape(8, 128).T),
        "mb2_d": np.ascontiguousarray(
            np.asarray(inputs["mlp_b2"], np.float32).reshape(8, 128).T),
        "mb3_d": np.ascontiguousarray(
            np.asarray(inputs["mlp_b3"], np.float32)[:, None]),
        "hb1_d": np.ascontiguousarray(
            np.asarray(inputs["head_b1"], np.float32).T),
        "hb2_d": np.ascontiguousarray(
            np.asarray(inputs["head_b2"], np.float32).T),
    }

    in_maps = []
    ones_row = np.ones((1, NL), np.float32)
    for c in range(NCORES):
        lo = NL * c
        mask = (dst >= lo) & (dst < lo + NL)
        flat = src[mask] * NL + (dst[mask] - lo)
        a = np.bincount(flat, minlength=N * NL).astype(np.float32)
        # paired layout for DoubleRow: [p, kb2, two, dst]
        a = a.reshape(KB2, 2, 128, NL).transpose(2, 0, 1, 3)
        m = dict(shared)
        m["acm8"] = np.ascontiguousarray(a).astype(ml_dtypes.float8_e4m3)
        m["xt"] = np.ascontiguousarray(
            np.vstack([x[lo:lo + NL].T, ones_row]))
        in_maps.append(m)
    return in_maps


def kernel(**inputs) -> np.ndarray:
    global _NC_CACHE
    if _NC_CACHE is None:
        _NC_CACHE = build_nc()
    nc = _NC_CACHE
    in_maps = _host_prep(inputs)
    res = run_bass_kernel_spmd(nc, in_maps, core_ids=list(range(NCORES)))
    out = np.concatenate(
        [np.asarray(res.results[c]["qout"]).reshape(GL, K, NG, NG)
         for c in range(NCORES)], axis=0)
    return out


# revision 15
# speedup vs baseline: 1.4388x; 1.1382x over previous
import sys
sys.path.insert(0, "/opt/trn_rl_repo")
import math
import numpy as np
import ml_dtypes

import concourse.bass as bass
from concourse import bacc, mybir
from concourse.tile import TileContext
from concourse.bass_utils import run_bass_kernel_spmd
from concourse.masks import make_identity

F32 = mybir.dt.float32
F32R = mybir.dt.float32r
F8 = mybir.dt.float8e4
AF = mybir.ActivationFunctionType
ALU = mybir.AluOpType
AX = mybir.AxisListType
DR = mybir.MatmulPerfMode.DoubleRow

N, G, E = 16384, 32, 524288
D, DFF, ZI, K, L = 512, 1024, 64, 4, 4
UMAP_A, UMAP_B = 1.577, 0.8951
BN_EPS = 1e-5
NCORES = 8
NL = N // NCORES      # 2048 local nodes per core
GL = G // NCORES      # 4 local graphs per core
NG = N // G           # 512 nodes per graph
KB2 = N // 256        # 64 paired source blocks (256 src nodes each)

_NC_CACHE = None


def build_nc():
    nc = bacc.Bacc("TRN2", target_bir_lowering=False, debug=False,
                   enable_asserts=True, num_devices=NCORES)

    xt = nc.dram_tensor("xt", (10, NL), F32R, kind="ExternalInput")
    # full-graph x in paired fp8: cols = [x_hi(10) | x_lo(10) | ones(1) | pad]
    xg8 = nc.dram_tensor("xg8", (128, KB2, 2, 32), F8, kind="ExternalInput")
    # emb_w stacked for the hi/lo recombine: rows = [W, W, b, 0, 0, 0]
    embwa = nc.dram_tensor("embwa", (32, D), F32R, kind="ExternalInput")
    acm8 = nc.dram_tensor("acm8", (128, KB2, 2, NL), F8, kind="ExternalInput")
    embw = nc.dram_tensor("embw", (10, D), F32R, kind="ExternalInput")
    gw1 = nc.dram_tensor("gw1", (L * D, D), F32R, kind="ExternalInput")
    gw2 = nc.dram_tensor("gw2", (L * D, D), F32R, kind="ExternalInput")
    mw1 = nc.dram_tensor("mw1", (D, DFF), F32R, kind="ExternalInput")
    mw2 = nc.dram_tensor("mw2", (DFF, DFF), F32R, kind="ExternalInput")
    mw3 = nc.dram_tensor("mw3", (DFF, ZI), F32R, kind="ExternalInput")
    hw1 = nc.dram_tensor("hw1", (K * ZI, ZI), F32R, kind="ExternalInput")
    hw2 = nc.dram_tensor("hw2", (K * ZI, ZI), F32R, kind="ExternalInput")
    gb1_d = nc.dram_tensor("gb1_d", (128, 16), F32, kind="ExternalInput")
    bng_d = nc.dram_tensor("bng_d", (128, 16), F32, kind="ExternalInput")
    bnb_d = nc.dram_tensor("bnb_d", (128, 16), F32, kind="ExternalInput")
    mb1_d = nc.dram_tensor("mb1_d", (128, 8), F32, kind="ExternalInput")
    mb2_d = nc.dram_tensor("mb2_d", (128, 8), F32, kind="ExternalInput")
    mb3_d = nc.dram_tensor("mb3_d", (ZI, 1), F32, kind="ExternalInput")
    hb1_d = nc.dram_tensor("hb1_d", (ZI, K), F32, kind="ExternalInput")
    hb2_d = nc.dram_tensor("hb2_d", (ZI, K), F32, kind="ExternalInput")
    qout = nc.dram_tensor("qout", (GL * K * 4 * 128, NG), F32,
                          kind="ExternalOutput")

    with TileContext(nc) as tc:
        with (
            tc.tile_pool(name="const", bufs=1) as cp,
            tc.tile_pool(name="res", bufs=1) as rp,
            tc.tile_pool(name="ps", bufs=1, space="PSUM") as ps,
            tc.tile_pool(name="dram", bufs=1, space="DRAM") as dp,
        ):
            ident = cp.tile([128, 128], F32, tag="ident")
            make_identity(nc, ident[:])
            nla = cp.tile([128, 1], F32, tag="nla")
            nc.gpsimd.memset(nla[:], -math.log(UMAP_A))
            of = cp.tile([64, 1], F32, tag="of")
            nc.gpsimd.memset(of[:], 1.0)
            onf = cp.tile([1, NG], F32, tag="onf")
            nc.gpsimd.memset(onf[:], 1.0)

            gb1w = cp.tile([128, 16], F32, tag="gb1w")
            nc.sync.dma_start(gb1w[:], gb1_d[:, :])
            bngw = cp.tile([128, 16], F32, tag="bngw")
            nc.sync.dma_start(bngw[:], bng_d[:, :])
            bnbw = cp.tile([128, 16], F32, tag="bnbw")
            nc.sync.dma_start(bnbw[:], bnb_d[:, :])
            mb1w = cp.tile([128, 8], F32, tag="mb1w")
            nc.sync.dma_start(mb1w[:], mb1_d[:, :])
            mb2w = cp.tile([128, 8], F32, tag="mb2w")
            nc.sync.dma_start(mb2w[:], mb2_d[:, :])
            mb3w = cp.tile([ZI, 1], F32, tag="mb3w")
            nc.sync.dma_start(mb3w[:], mb3_d[:, :])
            hb1w = cp.tile([ZI, K], F32, tag="hb1w")
            nc.sync.dma_start(hb1w[:], hb1_d[:, :])
            hb2w = cp.tile([ZI, K], F32, tag="hb2w")
            nc.sync.dma_start(hb2w[:], hb2_d[:, :])

            hT = [rp.tile([128, NL], F32R, tag=f"hT{fc}", name=f"hT{fc}")
                  for fc in range(4)]

# layout: [kb2][p][two][hilo*512+d] — hi and lo fp8 halves of h
            slice8 = [dp.tile([NL // 256, 128, 2, 2 * D], F8, tag=f"s8_{i}",
                              name=f"s8_{i}") for i in range(L)]
            table8 = [dp.tile([KB2, 128, 2, 2 * D], F8, tag=f"t8_{i}",
                              name=f"t8_{i}", addr_space="Shared")
                      for i in range(L)]
            bn_loc = [dp.tile([128, 8], F32, tag=f"bl{i}", name=f"bl{i}")
                      for i in range(L)]
            bn_glob = [dp.tile([128, 8], F32, tag=f"bg{i}", name=f"bg{i}",
                               addr_space="Shared")
                       for i in range(L)]

            # ---------------- embedding (D-major hT only) ----------------
            with tc.tile_pool(name="emb", bufs=1) as ep:
                xt_sb = ep.tile([10, NL], F32R, tag="xt")
                nc.sync.dma_start(xt_sb[:], xt[:, :])
                ew_sb = ep.tile([10, D], F32R, tag="ew")
                nc.sync.dma_start(ew_sb[:], embw[:, :])
                for fc in range(4):
                    for j in range(4):
                        p = ps.tile([128, 512], F32, tag=f"b{4 + fc}")
                        nc.tensor.matmul(p[:], ew_sb[:, 128 * fc:128 * fc + 128],
                                         xt_sb[:, 512 * j:512 * j + 512],
                                         start=True, stop=True)
                        nc.vector.tensor_copy(hT[fc][:, 512 * j:512 * j + 512],
                                              p[:])

            # ---------------- GIN layers ----------------
            with tc.tile_pool(name="gin", bufs=1) as gp:
                for l in range(L):
                    w1s = gp.tile([128, 2048], F32R, tag="w1")
                    w2s = gp.tile([128, 2048], F32R, tag="w2")
                    for ic in range(4):
                        r0 = 512 * l + 128 * ic
                        nc.sync.dma_start(w1s[:, 512 * ic:512 * ic + 512],
                                          gw1[r0:r0 + 128, :])
                        nc.sync.dma_start(w2s[:, 512 * ic:512 * ic + 512],
                                          gw2[r0:r0 + 128, :])

                    mt = [gp.tile([128, NL], F32R, tag=f"mt{fc}",
                                  name=f"mt{fc}_{l}") for fc in range(4)]

                    if l == 0:
                        # layer-0 aggregation, exact: agg0 = (A.T xa) @ Wa
                        # xa = [x_hi | x_lo | 1], Wa = [W; W; b]
                        xg_sb = gp.tile([128, KB2, 2, 32], F8, tag="xg")
                        nc.sync.dma_start(xg_sb[:], xg8[:, :, :, :])
                        ewa = gp.tile([32, D], F32R, tag="ewa")
                        nc.sync.dma_start(ewa[:], embwa[:, :])
                        px = [ps.tile([32, 512], F32, tag=f"b{i}",
                                      name=f"px{i}") for i in range(4)]
                        for kb2 in range(KB2):
                            a8 = gp.tile([128, 2, 2048], F8, tag="a8f", bufs=3)
                            nc.sync.dma_start(a8[:], acm8[:, kb2, :, :])
                            lhs = xg_sb[:, kb2, :, :]
                            for dt in range(4):
                                nc.tensor.matmul(
                                    px[dt][:], lhs,
                                    a8[:, :, 512 * dt:512 * dt + 512],
                                    start=(kb2 == 0), stop=(kb2 == KB2 - 1),
                                    perf_mode=DR)
                        atxt = gp.tile([32, NL], F32R, tag="atxt")
                        for dt in range(4):
                            nc.vector.tensor_copy(
                                atxt[:, 512 * dt:512 * dt + 512], px[dt][:])
                        for d in range(4):
                            for jc in range(4):
                                pr0 = ps.tile([128, 512], F32,
                                              tag=f"b{4 + jc % 4}")
                                nc.tensor.matmul(
                                    pr0[:], ewa[:, 128 * d:128 * d + 128],
                                    atxt[:, 512 * jc:512 * jc + 512],
                                    start=True, stop=True)
                                nc.vector.tensor_tensor(
                                    out=mt[d][:, 512 * jc:512 * jc + 512],
                                    in0=pr0[:],
                                    in1=hT[d][:, 512 * jc:512 * jc + 512],
                                    op=ALU.add)
                    else:
                        # aggregation: aggT = (hi + lo).T @ A8 (DoubleRow) + hT
                        for half in range(2):
                            pb = [ps.tile([128, 512], F32, tag=f"b{i}",
                                          name=f"pb{i}") for i in range(8)]
                            for kb2 in range(KB2):
                                a8 = gp.tile([128, 2, 1024], F8, tag="a8",
                                             bufs=4)
                                nc.sync.dma_start(
                                    a8[:],
                                    acm8[:, kb2, :,
                                         1024 * half:1024 * half + 1024])
                                hk8 = gp.tile([128, 2, 1024], F8, tag="hk8",
                                              bufs=4)
                                nc.sync.dma_start(hk8[:],
                                                  table8[l][kb2, :, :, :])
                                for d in range(4):
                                    for hilo in range(2):
                                        lhs = hk8[:, :,
                                                  512 * hilo + 128 * d:
                                                  512 * hilo + 128 * d + 128]
                                        for dc in range(2):
                                            nc.tensor.matmul(
                                                pb[2 * d + dc][:], lhs,
                                                a8[:, :,
                                                   512 * dc:512 * dc + 512],
                                                start=(kb2 == 0 and hilo == 0),
                                                stop=(kb2 == KB2 - 1
                                                      and hilo == 1),
                                                perf_mode=DR)
                            for d in range(4):
                                for dc in range(2):
                                    col = 1024 * half + 512 * dc
                                    nc.vector.tensor_tensor(
                                        out=mt[d][:, col:col + 512],
                                        in0=pb[2 * d + dc][:],
                                        in1=hT[d][:, col:col + 512],
                                        op=ALU.add)

                    # GIN MLP: u1 = relu(m@w1+b1); u2 = u1@w2 (into mt)
                    for j in range(4):
                        ncol = 512 * j
                        u1c = [gp.tile([128, 512], F32R, tag=f"u1_{oc}", bufs=2,
                                       name=f"u1c{oc}") for oc in range(4)]
                        for oc in range(4):
                            p = ps.tile([128, 512], F32, tag=f"b{oc}")
                            for ic in range(4):
                                nc.tensor.matmul(
                                    p[:],
                                    w1s[:, 512 * ic + 128 * oc:
                                        512 * ic + 128 * oc + 128],
                                    mt[ic][:, ncol:ncol + 512],
                                    start=(ic == 0), stop=(ic == 3))
                            nc.scalar.activation(
                                u1c[oc][:], p[:], AF.Relu,
                                bias=gb1w[:, 4 * l + oc:4 * l + oc + 1])
                        for oc in range(4):
                            p = ps.tile([128, 512], F32, tag=f"b{4 + oc}")
                            for ic in range(4):
                                nc.tensor.matmul(
                                    p[:],
                                    w2s[:, 512 * ic + 128 * oc:
                                        512 * ic + 128 * oc + 128],
                                    u1c[ic][:],
                                    start=(ic == 0), stop=(ic == 3))
                            nc.vector.tensor_copy(mt[oc][:, ncol:ncol + 512],
                                                  p[:])

                    # BN stats (local sums) -> AllReduce
                    stat = gp.tile([128, 8], F32, tag="stat")
                    for fc in range(4):
                        nc.vector.reduce_sum(stat[:, fc:fc + 1], mt[fc][:],
                                             axis=AX.X)
                        qacc = gp.tile([128, 1], F32, tag="qacc")
                        for j in range(4):
                            sq = gp.tile([128, 512], F32, tag="sq", bufs=2)
                            nc.scalar.activation(
                                sq[:], mt[fc][:, 512 * j:512 * j + 512],
                                AF.Square)
                            qp = gp.tile([128, 1], F32, tag=f"qp{j}")
                            nc.vector.reduce_sum(qp[:], sq[:], axis=AX.X)
                            if j == 0:
                                nc.vector.tensor_copy(qacc[:], qp[:])
                            else:
                                nc.vector.tensor_tensor(
                                    out=qacc[:], in0=qp[:], in1=qacc[:],
                                    op=ALU.add)
                        nc.vector.tensor_copy(stat[:, 4 + fc:5 + fc], qacc[:])
                    nc.sync.dma_start(bn_loc[l][:, :], stat[:])
                    nc.gpsimd.collective_compute(
                        "AllReduce", ALU.add,
                        ins=[bn_loc[l][:, :].opt()],
                        outs=[bn_glob[l][:, :].opt()],
                        replica_groups=[list(range(NCORES))],
                    )
                    ga = gp.tile([128, 8], F32, tag="ga")
                    nc.sync.dma_start(ga[:], bn_glob[l][:, :])

                    # BN apply + relu + residual (in place into hT)
                    for fc in range(4):
                        mu = gp.tile([128, 1], F32, tag="mu")
                        nc.vector.tensor_scalar(out=mu[:], in0=ga[:, fc:fc + 1],
                                                scalar1=1.0 / N, scalar2=None,
                                                op0=ALU.mult)
                        ex2 = gp.tile([128, 1], F32, tag="ex2")
                        nc.vector.tensor_scalar(out=ex2[:],
                                                in0=ga[:, 4 + fc:5 + fc],
                                                scalar1=1.0 / N, scalar2=None,
                                                op0=ALU.mult)
                        mu2 = gp.tile([128, 1], F32, tag="mu2")
                        nc.vector.tensor_tensor(out=mu2[:], in0=mu[:],
                                                in1=mu[:], op=ALU.mult)
                        var = gp.tile([128, 1], F32, tag="var")
                        nc.vector.tensor_tensor(out=var[:], in0=ex2[:],
                                                in1=mu2[:], op=ALU.subtract)
                        vare = gp.tile([128, 1], F32, tag="vare")
                        nc.vector.tensor_scalar(out=vare[:], in0=var[:],
                                                scalar1=BN_EPS, scalar2=None,
                                                op0=ALU.add)
                        std = gp.tile([128, 1], F32, tag="std")
                        nc.scalar.activation(std[:], vare[:], AF.Sqrt)
                        inv = gp.tile([128, 1], F32, tag="inv")
                        nc.vector.reciprocal(inv[:], std[:])
                        sv = gp.tile([128, 1], F32, tag="sv")
                        nc.vector.tensor_tensor(
                            out=sv[:], in0=inv[:],
                            in1=bngw[:, 4 * l + fc:4 * l + fc + 1],
                            op=ALU.mult)
                        mst = gp.tile([128, 1], F32, tag="mst")
                        nc.vector.tensor_tensor(out=mst[:], in0=mu[:],
                                                in1=sv[:], op=ALU.mult)
                        tv = gp.tile([128, 1], F32, tag="tv")
                        nc.vector.tensor_tensor(
                            out=tv[:], in0=bnbw[:, 4 * l + fc:4 * l + fc + 1],
                            in1=mst[:], op=ALU.subtract)
                        for j in range(4):
                            ncol = 512 * j
                            rt = gp.tile([128, 512], F32R, tag="rt", bufs=2)
                            nc.scalar.activation(
                                rt[:], mt[fc][:, ncol:ncol + 512], AF.Relu,
                                bias=tv[:, 0:1], scale=sv[:, 0:1])
                            nc.vector.tensor_tensor(
                                out=hT[fc][:, ncol:ncol + 512], in0=rt[:],
                                in1=hT[fc][:, ncol:ncol + 512], op=ALU.add)

                    # write updated h (fp8) back to the replicated table
                    if l < L - 1:
                        for nb in range(16):
                            hn2 = gp.tile([128, 2 * 512], F8, tag="hn2",
                                          bufs=2)
                            for fc in range(4):
                                pt = ps.tile([128, 128], F32, tag=f"b{fc}")
                                nc.tensor.transpose(
                                    pt[:],
                                    hT[fc][:, 128 * nb:128 * nb + 128]
                                    .bitcast(F32),
                                    ident[:])
                                nc.vector.tensor_copy(
                                    hn2[:, 128 * fc:128 * fc + 128], pt[:])
                                h32 = gp.tile([128, 128], F32, tag="h32",
                                              bufs=2)
                                nc.vector.tensor_copy(
                                    h32[:], hn2[:, 128 * fc:128 * fc + 128])
                                nc.vector.tensor_tensor(
                                    out=hn2[:, 512 + 128 * fc:
                                            512 + 128 * fc + 128],
                                    in0=pt[:], in1=h32[:], op=ALU.subtract)
                            nc.sync.dma_start(
                                slice8[l + 1][nb // 2, :, nb % 2, :], hn2[:])
                        nc.gpsimd.collective_compute(
                            "AllGather", ALU.bypass,
                            ins=[slice8[l + 1][:, :, :, :].opt()],
                            outs=[table8[l + 1][:, :, :, :].opt()],
                            replica_groups=[list(range(NCORES))],
                        )

            # ---------------- final MLP + heads + pairwise ----------------
            with tc.tile_pool(name="fin", bufs=1) as fp:
                mwa = [fp.tile([128, DFF], F32R, tag=f"mw1_{ic}", name=f"mwa{ic}")
                       for ic in range(4)]
                for ic in range(4):
                    nc.sync.dma_start(mwa[ic][:],
                                      mw1[128 * ic:128 * ic + 128, :])
                mwb = [fp.tile([128, DFF], F32R, tag=f"mw2_{ic}", name=f"mwb{ic}")
                       for ic in range(8)]
                for ic in range(8):
                    nc.sync.dma_start(mwb[ic][:],
                                      mw2[128 * ic:128 * ic + 128, :])
                mwc = [fp.tile([128, ZI], F32R, tag=f"mw3_{ic}", name=f"mwc{ic}")
                       for ic in range(8)]
                for ic in range(8):
                    nc.sync.dma_start(mwc[ic][:],
                                      mw3[128 * ic:128 * ic + 128, :])
                hw1s = [fp.tile([ZI, ZI], F32R, tag=f"hw1_{k}", name=f"hw1s{k}")
                        for k in range(K)]
                hw2s = [fp.tile([ZI, ZI], F32R, tag=f"hw2_{k}", name=f"hw2s{k}")
                        for k in range(K)]
                for k in range(K):
                    nc.sync.dma_start(hw1s[k][:], hw1[ZI * k:ZI * k + ZI, :])
                    nc.sync.dma_start(hw2s[k][:], hw2[ZI * k:ZI * k + ZI, :])

                for g in range(GL):
                    gcol = 512 * g
                    z1 = [fp.tile([128, 512], F32R, tag=f"z1_{oc}", name=f"z1_{oc}")
                          for oc in range(8)]
                    for oc in range(8):
                        p = ps.tile([128, 512], F32, tag=f"b{oc}")
                        for ic in range(4):
                            nc.tensor.matmul(
                                p[:],
                                mwa[ic][:, 128 * oc:128 * oc + 128],
                                hT[ic][:, gcol:gcol + 512],
                                start=(ic == 0), stop=(ic == 3))
                        nc.scalar.activation(z1[oc][:], p[:], AF.Relu,
                                             bias=mb1w[:, oc:oc + 1])
                    z2 = [fp.tile([128, 512], F32R, tag=f"z2_{oc}", name=f"z2_{oc}")
                          for oc in range(8)]
                    for oc in range(8):
                        p = ps.tile([128, 512], F32, tag=f"b{oc}")
                        for ic in range(8):
                            nc.tensor.matmul(
                                p[:],
                                mwb[ic][:, 128 * oc:128 * oc + 128],
                                z1[ic][:],
                                start=(ic == 0), stop=(ic == 7))
                        nc.scalar.activation(z2[oc][:], p[:], AF.Relu,
                                             bias=mb2w[:, oc:oc + 1])
                    pz = ps.tile([ZI, 512], F32, tag="b0")
                    for ic in range(8):
                        nc.tensor.matmul(pz[:], mwc[ic][:, 0:ZI], z2[ic][:],
                                         start=(ic == 0), stop=(ic == 7))
                    z3 = fp.tile([ZI, 512], F32R, tag="z3")
                    nc.vector.tensor_tensor(
                        out=z3[:], in0=pz[:],
                        in1=mb3w[:, 0:1].to_broadcast([ZI, 512])[:],
                        op=ALU.add)
                    for k in range(K):
                        p1 = ps.tile([ZI, 512], F32, tag="b1")
                        nc.tensor.matmul(p1[:], hw1s[k][:], z3[:],
                                         start=True, stop=True)
                        h1 = fp.tile([ZI, 512], F32R, tag="h1", bufs=2)
                        nc.scalar.activation(h1[:], p1[:], AF.Relu,
                                             bias=hb1w[:, k:k + 1])
                        p2 = ps.tile([ZI, 512], F32, tag="b2")
                        nc.tensor.matmul(p2[:], hw2s[k][:], h1[:],
                                         start=True, stop=True)
                        hkt = fp.tile([ZI, 512], F32, tag="hkt", bufs=2)
                        nc.vector.tensor_tensor(
                            out=hkt[:], in0=p2[:],
                            in1=hb2w[:, k:k + 1].to_broadcast([ZI, 512])[:],
                            op=ALU.add)
                        hm2 = fp.tile([ZI, 512], F32, tag="hm2", bufs=2)
                        nc.vector.tensor_scalar(out=hm2[:], in0=hkt[:],
                                                scalar1=-2.0, scalar2=None,
                                                op0=ALU.mult)
                        sqt = fp.tile([ZI, 512], F32, tag="sqt", bufs=2)
                        nc.vector.tensor_tensor(out=sqt[:], in0=hkt[:],
                                                in1=hkt[:], op=ALU.mult)
                        pr = ps.tile([1, 512], F32, tag="b3")
                        nc.tensor.matmul(pr[:], of[:], sqt[:],
                                         start=True, stop=True)
                        rsb = fp.tile([1, 512], F32, tag="rsb", bufs=2)
                        nc.vector.tensor_copy(rsb[:], pr[:])
                        for mb in range(4):
                            pd = ps.tile([128, 512], F32, tag=f"b{4 + mb}")
                            nc.tensor.matmul(pd[:],
                                             hm2[:, 128 * mb:128 * mb + 128],
                                             hkt[:], start=True, stop=False)
                            nc.tensor.matmul(pd[:], onf[:, 0:128], rsb[:],
                                             start=False, stop=False,
                                             skip_group_check=True)
                            nc.tensor.matmul(pd[:],
                                             rsb[:, 128 * mb:128 * mb + 128],
                                             onf[:], start=False, stop=True,
                                             skip_group_check=True)
                            d2t = fp.tile([128, 512], F32, tag="d2", bufs=2)
                            nc.vector.tensor_scalar(out=d2t[:], in0=pd[:],
                                                    scalar1=1e-12,
                                                    scalar2=None, op0=ALU.max)
                            lnt = fp.tile([128, 512], F32, tag="ln", bufs=2)
                            nc.scalar.activation(lnt[:], d2t[:], AF.Ln)
                            qt = fp.tile([128, 512], F32, tag="qt", bufs=3)
                            nc.scalar.activation(qt[:], lnt[:], AF.Sigmoid,
                                                 bias=nla[:, 0:1],
                                                 scale=-UMAP_B)
                            row = ((g * K + k) * 4 + mb) * 128
                            nc.sync.dma_start(qout[row:row + 128, :], qt[:])
    nc.compile()
    return nc


def _host_prep(inputs):
    x = np.asarray(inputs["x"], np.float32)
    edge_index = np.asarray(inputs["edge_index"], np.int64)
    src, dst = edge_index[0], edge_index[1]

    shared = {
        "embw": np.ascontiguousarray(np.vstack(
            [np.asarray(inputs["emb_w"], np.float32),
             np.asarray(inputs["emb_b"], np.float32)[None, :]])),
        "gw1": np.ascontiguousarray(
            np.asarray(inputs["gin_w1"], np.float32).reshape(L * D, D)),
        "gw2": np.ascontiguousarray(
            np.asarray(inputs["gin_w2"], np.float32).reshape(L * D, D)),
        "mw1": np.ascontiguousarray(np.asarray(inputs["mlp_w1"], np.float32)),
        "mw2": np.ascontiguousarray(np.asarray(inputs["mlp_w2"], np.float32)),
        "mw3": np.ascontiguousarray(np.asarray(inputs["mlp_w3"], np.float32)),
        "hw1": np.ascontiguousarray(
            np.asarray(inputs["head_w1"], np.float32).reshape(K * ZI, ZI)),
        "hw2": np.ascontiguousarray(
            np.asarray(inputs["head_w2"], np.float32).reshape(K * ZI, ZI)),
        "gb1_d": np.ascontiguousarray(
            np.asarray(inputs["gin_b1"], np.float32)
            .reshape(L, 4, 128).transpose(2, 0, 1).reshape(128, 16)),
        "bng_d": np.ascontiguousarray(
            np.asarray(inputs["bn_g"], np.float32)
            .reshape(L, 4, 128).transpose(2, 0, 1).reshape(128, 16)),
        "bnb_d": np.ascontiguousarray(
            np.asarray(inputs["bn_b"], np.float32)
            .reshape(L, 4, 128).transpose(2, 0, 1).reshape(128, 16)),
        "mb1_d": np.ascontiguousarray(
            np.asarray(inputs["mlp_b1"], np.float32).reshape(8, 128).T),
        "mb2_d": np.ascontiguousarray(
            np.asarray(inputs["mlp_b2"], np.float32).reshape(8, 128).T),
        "mb3_d": np.ascontiguousarray(
            np.asarray(inputs["mlp_b3"], np.float32)[:, None]),
        "hb1_d": np.ascontiguousarray(
            np.asarray(inputs["head_b1"], np.float32).T),
        "hb2_d": np.ascontiguousarray(
            np.asarray(inputs["head_b2"], np.float32).T),
    }

    # layer-0 exact aggregation operands
    emb_w = np.asarray(inputs["emb_w"], np.float32)
    emb_b = np.asarray(inputs["emb_b"], np.float32)
    xhi = x.astype(ml_dtypes.float8_e4m3)
    xlo = (x - xhi.astype(np.float32)).astype(ml_dtypes.float8_e4m3)
    xa = np.zeros((N, 32), ml_dtypes.float8_e4m3)
    xa[:, 0:9] = xhi
    xa[:, 10:19] = xlo
    xa[:, 20] = np.float32(1.0)
    # paired layout [p, kb2, two, col]
    shared["xg8"] = np.ascontiguousarray(
        xa.reshape(KB2, 2, 128, 32).transpose(2, 0, 1, 3))
    ewa = np.zeros((32, D), np.float32)
    ewa[0:9] = emb_w
    ewa[10:19] = emb_w
    ewa[20] = emb_b
    shared["embwa"] = ewa

    in_maps = []
    ones_row = np.ones((1, NL), np.float32)
    for c in range(NCORES):
        lo = NL * c
        mask = (dst >= lo) & (dst < lo + NL)
        flat = src[mask] * NL + (dst[mask] - lo)
        a = np.bincount(flat, minlength=N * NL).astype(np.float32)
        # paired layout for DoubleRow: [p, kb2, two, dst]
        a = a.reshape(KB2, 2, 128, NL).transpose(2, 0, 1, 3)
        m = dict(shared)
        m["acm8"] = np.ascontiguousarray(a).astype(ml_dtypes.float8_e4m3)
        m["xt"] = np.ascontiguousarray(
            np.vstack([x[lo:lo + NL].T, ones_row]))
        in_maps.append(m)
    return in_maps


def kernel(**inputs) -> np.ndarray:
    global _NC_CACHE
    if _NC_CACHE is None:
        _NC_CACHE = build_nc()
    nc = _NC_CACHE
    in_maps = _host_prep(inputs)
    res = run_bass_kernel_spmd(nc, in_maps, core_ids=list(range(NCORES)))
    out = np.concatenate(
        [np.asarray(res.results[c]["qout"]).reshape(GL, K, NG, NG)
         for c in range(NCORES)], axis=0)
    return out


# revision 20
# speedup vs baseline: 1.5282x; 1.0621x over previous
import sys
sys.path.insert(0, "/opt/trn_rl_repo")
import math
import numpy as np
import ml_dtypes

import concourse.bass as bass
from concourse import bacc, mybir
from concourse.tile import TileContext
from concourse.bass_utils import run_bass_kernel_spmd
from concourse.masks import make_identity

F32 = mybir.dt.float32
F32R = mybir.dt.float32r
F8 = mybir.dt.float8e4
AF = mybir.ActivationFunctionType
ALU = mybir.AluOpType
AX = mybir.AxisListType
DR = mybir.MatmulPerfMode.DoubleRow

N, G, E = 16384, 32, 524288
D, DFF, ZI, K, L = 512, 1024, 64, 4, 4
UMAP_A, UMAP_B = 1.577, 0.8951
BN_EPS = 1e-5
NCORES = 8
NL = N // NCORES      # 2048 local nodes per core
GL = G // NCORES      # 4 local graphs per core
NG = N // G           # 512 nodes per graph
KB2 = N // 256        # 64 paired source blocks (256 src nodes each)
RG = [list(range(NCORES))]

_NC_CACHE = None


def build_nc():
    nc = bacc.Bacc("TRN2", target_bir_lowering=False, debug=False,
                   enable_asserts=True, num_devices=NCORES)

    xt = nc.dram_tensor("xt", (10, NL), F32R, kind="ExternalInput")
    # full-graph x in paired fp8: cols = [x_hi(10) | x_lo(10) | ones(1) | pad]
    xg8 = nc.dram_tensor("xg8", (128, KB2, 2, 32), F8, kind="ExternalInput")
    # emb_w stacked for the hi/lo recombine: rows = [W, W, b, 0...]
    embwa = nc.dram_tensor("embwa", (32, D), F32R, kind="ExternalInput")
    acm8 = nc.dram_tensor("acm8", (128, KB2, 2, NL), F8, kind="ExternalInput")
    embw = nc.dram_tensor("embw", (10, D), F32R, kind="ExternalInput")
    gw1 = nc.dram_tensor("gw1", (L * D, D), F32R, kind="ExternalInput")
    gw2 = nc.dram_tensor("gw2", (L * D, D), F32R, kind="ExternalInput")
    mw1 = nc.dram_tensor("mw1", (D, DFF), F32R, kind="ExternalInput")
    mw2 = nc.dram_tensor("mw2", (DFF, DFF), F32R, kind="ExternalInput")
    mw3 = nc.dram_tensor("mw3", (DFF, ZI), F32R, kind="ExternalInput")
    # head weights batched in pairs: hw1b[64*kp:64*kp+64] = [w1_{2kp}|w1_{2kp+1}]
    hw1b = nc.dram_tensor("hw1b", (2 * ZI, 2 * ZI), F32R, kind="ExternalInput")
    # hw2b[128*kp:...] = blockdiag(w2_{2kp}, w2_{2kp+1})
    hw2b = nc.dram_tensor("hw2b", (2 * 2 * ZI, 2 * ZI), F32R,
                          kind="ExternalInput")
    gb1_d = nc.dram_tensor("gb1_d", (128, 16), F32, kind="ExternalInput")
    bng_d = nc.dram_tensor("bng_d", (128, 16), F32, kind="ExternalInput")
    bnb_d = nc.dram_tensor("bnb_d", (128, 16), F32, kind="ExternalInput")
    mb1_d = nc.dram_tensor("mb1_d", (128, 8), F32, kind="ExternalInput")
    mb2_d = nc.dram_tensor("mb2_d", (128, 8), F32, kind="ExternalInput")
    mb3_d = nc.dram_tensor("mb3_d", (ZI, 1), F32, kind="ExternalInput")
    hb1b_d = nc.dram_tensor("hb1b_d", (128, 2), F32, kind="ExternalInput")
    hb2b_d = nc.dram_tensor("hb2b_d", (128, 2), F32, kind="ExternalInput")
    selm_d = nc.dram_tensor("selm_d", (128, 2), F32, kind="ExternalInput")
    qout = nc.dram_tensor("qout", (GL * K * 4 * 128, NG), F32,
                          kind="ExternalOutput")

    with TileContext(nc) as tc:
        with (
            tc.tile_pool(name="const", bufs=1) as cp,
            tc.tile_pool(name="res", bufs=1) as rp,
            tc.tile_pool(name="ps", bufs=1, space="PSUM") as ps,
            tc.tile_pool(name="dram", bufs=1, space="DRAM") as dp,
        ):
            ident = cp.tile([128, 128], F32, tag="ident")
            make_identity(nc, ident[:])
            nla = cp.tile([128, 1], F32, tag="nla")
            nc.gpsimd.memset(nla[:], -math.log(UMAP_A))
            onf = cp.tile([1, NG], F32, tag="onf")
            nc.gpsimd.memset(onf[:], 1.0)

            gb1w = cp.tile([128, 16], F32, tag="gb1w")
            nc.sync.dma_start(gb1w[:], gb1_d[:, :])
            bngw = cp.tile([128, 16], F32, tag="bngw")
            nc.sync.dma_start(bngw[:], bng_d[:, :])
            bnbw = cp.tile([128, 16], F32, tag="bnbw")
            nc.sync.dma_start(bnbw[:], bnb_d[:, :])
            mb1w = cp.tile([128, 8], F32, tag="mb1w")
            nc.sync.dma_start(mb1w[:], mb1_d[:, :])
            mb2w = cp.tile([128, 8], F32, tag="mb2w")
            nc.sync.dma_start(mb2w[:], mb2_d[:, :])
            mb3w = cp.tile([ZI, 1], F32, tag="mb3w")
            nc.sync.dma_start(mb3w[:], mb3_d[:, :])
            hb1w = cp.tile([128, 2], F32, tag="hb1w")
            nc.sync.dma_start(hb1w[:], hb1b_d[:, :])
            hb2w = cp.tile([128, 2], F32, tag="hb2w")
            nc.sync.dma_start(hb2w[:], hb2b_d[:, :])
            selm = cp.tile([128, 2], F32, tag="selm")
            nc.sync.dma_start(selm[:], selm_d[:, :])

            hT = [rp.tile([128, NL], F32R, tag=f"hT{fc}", name=f"hT{fc}")
                  for fc in range(4)]

            # chunked hi/lo tables: A covers local kb2 0-3, B covers 4-7
            sA = [dp.tile([4, 128, 2, 2 * D], F8, tag=f"sA{i}", name=f"sA{i}")
                  for i in range(L)]
            sB = [dp.tile([4, 128, 2, 2 * D], F8, tag=f"sB{i}", name=f"sB{i}")
                  for i in range(L)]
            tA = [dp.tile([KB2 // 2, 128, 2, 2 * D], F8, tag=f"tA{i}",
                          name=f"tA{i}", addr_space="Shared")
                  for i in range(L)]
            tB = [dp.tile([KB2 // 2, 128, 2, 2 * D], F8, tag=f"tB{i}",
                          name=f"tB{i}", addr_space="Shared")
                  for i in range(L)]
            bn_loc = [dp.tile([128, 8], F32, tag=f"bl{i}", name=f"bl{i}")
                      for i in range(L)]
            bn_glob = [dp.tile([128, 8], F32, tag=f"bg{i}", name=f"bg{i}",
                               addr_space="Shared")
                       for i in range(L)]
            war_l = dp.tile([128, 8], F32, name="war_l")
            war_g = dp.tile([128, 8], F32, name="war_g", addr_space="Shared")

            # ---------------- embedding + collective warmup ----------------
            with tc.tile_pool(name="emb", bufs=1) as ep:
                wz = ep.tile([128, 8], F32, tag="wz")
                nc.gpsimd.memset(wz[:], 0.0)
                nc.sync.dma_start(war_l[:, :], wz[:])
                nc.gpsimd.collective_compute(
                    "AllReduce", ALU.add, ins=[war_l[:, :].opt()],
                    outs=[war_g[:, :].opt()], replica_groups=RG)
                xt_sb = ep.tile([10, NL], F32R, tag="xt")
                nc.sync.dma_start(xt_sb[:], xt[:, :])
                ew_sb = ep.tile([10, D], F32R, tag="ew")
                nc.sync.dma_start(ew_sb[:], embw[:, :])
                for fc in range(4):
                    for j in range(4):
                        p = ps.tile([128, 512], F32, tag=f"b{4 + fc}")
                        nc.tensor.matmul(p[:], ew_sb[:, 128 * fc:128 * fc + 128],
                                         xt_sb[:, 512 * j:512 * j + 512],
                                         start=True, stop=True)
                        nc.vector.tensor_copy(hT[fc][:, 512 * j:512 * j + 512],
                                              p[:])

            # ---------------- GIN layers ----------------
            with tc.tile_pool(name="gin", bufs=1) as gp:
                for l in range(L):
                    w1s = gp.tile([128, 2048], F32R, tag="w1")
                    w2s = gp.tile([128, 2048], F32R, tag="w2")
                    for ic in range(4):
                        r0 = 512 * l + 128 * ic
                        nc.sync.dma_start(w1s[:, 512 * ic:512 * ic + 512],
                                          gw1[r0:r0 + 128, :])
                        nc.sync.dma_start(w2s[:, 512 * ic:512 * ic + 512],
                                          gw2[r0:r0 + 128, :])

                    mt = [gp.tile([128, NL], F32R, tag=f"mt{fc}",
                                  name=f"mt{fc}_{l}") for fc in range(4)]

                    if l == 0:
                        # layer-0 aggregation, exact: agg0 = (A.T xa) @ Wa
                        xg_sb = gp.tile([128, KB2, 2, 32], F8, tag="xg")
                        nc.sync.dma_start(xg_sb[:], xg8[:, :, :, :])
                        ewa = gp.tile([32, D], F32R, tag="ewa")
                        nc.sync.dma_start(ewa[:], embwa[:, :])
                        px = [ps.tile([32, 512], F32, tag=f"b{i}",
                                      name=f"px{i}") for i in range(4)]
                        for half in range(2):
                            for kb2 in range(KB2):
                                a8 = gp.tile([128, 2, 1024], F8, tag="a8",
                                             bufs=4)
                                eng = nc.sync if kb2 % 2 == 0 else nc.scalar
                                eng.dma_start(
                                    a8[:],
                                    acm8[:, kb2, :,
                                         1024 * half:1024 * half + 1024])
                                lhs = xg_sb[:, kb2, :, :]
                                for dc in range(2):
                                    nc.tensor.matmul(
                                        px[2 * half + dc][:], lhs,
                                        a8[:, :, 512 * dc:512 * dc + 512],
                                        start=(kb2 == 0), stop=(kb2 == KB2 - 1),
                                        perf_mode=DR)
                        atxt = gp.tile([32, NL], F32R, tag="atxt")
                        for dt in range(4):
                            nc.vector.tensor_copy(
                                atxt[:, 512 * dt:512 * dt + 512], px[dt][:])
                        for d in range(4):
                            for jc in range(4):
                                pr0 = ps.tile([128, 512], F32, tag=f"b{4 + jc}")
                                nc.tensor.matmul(
                                    pr0[:], ewa[:, 128 * d:128 * d + 128],
                                    atxt[:, 512 * jc:512 * jc + 512],
                                    start=True, stop=True)
                                nc.vector.tensor_tensor(
                                    out=mt[d][:, 512 * jc:512 * jc + 512],
                                    in0=pr0[:],
                                    in1=hT[d][:, 512 * jc:512 * jc + 512],
                                    op=ALU.add)
                    else:
                        # aggregation: aggT = (hi + lo).T @ A8 (DoubleRow) + hT
                        # consume chunk-A table rows first, then chunk-B
                        order = ([("A", r) for r in range(KB2 // 2)]
                                 + [("B", r) for r in range(KB2 // 2)])
                        for half in range(2):
                            pb = [ps.tile([128, 512], F32, tag=f"b{i}",
                                          name=f"pb{i}") for i in range(8)]
                            for ii, (ab, r) in enumerate(order):
                                gkb2 = 8 * (r // 4) + (r % 4) + (4 if ab == "B"
                                                                 else 0)
                                a8 = gp.tile([128, 2, 1024], F8, tag="a8",
                                             bufs=4)
                                nc.sync.dma_start(
                                    a8[:],
                                    acm8[:, gkb2, :,
                                         1024 * half:1024 * half + 1024])
                                hk8 = gp.tile([128, 2, 1024], F8, tag="hk8",
                                              bufs=4)
                                tab = tA[l] if ab == "A" else tB[l]
                                nc.scalar.dma_start(hk8[:], tab[r, :, :, :])
                                for d in range(4):
                                    for hilo in range(2):
                                        lhs = hk8[:, :,
                                                  512 * hilo + 128 * d:
                                                  512 * hilo + 128 * d + 128]
                                        for dc in range(2):
                                            nc.tensor.matmul(
                                                pb[2 * d + dc][:], lhs,
                                                a8[:, :,
                                                   512 * dc:512 * dc + 512],
                                                start=(ii == 0 and hilo == 0),
                                                stop=(ii == KB2 - 1
                                                      and hilo == 1),
                                                perf_mode=DR)
                            for d in range(4):
                                for dc in range(2):
                                    col = 1024 * half + 512 * dc
                                    nc.vector.tensor_tensor(
                                        out=mt[d][:, col:col + 512],
                                        in0=pb[2 * d + dc][:],
                                        in1=hT[d][:, col:col + 512],
                                        op=ALU.add)

                    # GIN MLP: u1 = relu(m@w1+b1); u2 = u1@w2 (into mt)
                    # fold BN partial stats in as u2 chunks are produced
                    st16s = gp.tile([128, 16], F32, tag="st16s")
                    st16q = gp.tile([128, 16], F32, tag="st16q")
                    for j in range(4):
                        ncol = 512 * j
                        u1c = [gp.tile([128, 512], F32R, tag=f"u1_{oc}", bufs=2,
                                       name=f"u1c{oc}") for oc in range(4)]
                        for oc in range(4):
                            p = ps.tile([128, 512], F32, tag=f"b{oc}")
                            for ic in range(4):
                                nc.tensor.matmul(
                                    p[:],
                                    w1s[:, 512 * ic + 128 * oc:
                                        512 * ic + 128 * oc + 128],
                                    mt[ic][:, ncol:ncol + 512],
                                    start=(ic == 0), stop=(ic == 3))
                            nc.scalar.activation(
                                u1c[oc][:], p[:], AF.Relu,
                                bias=gb1w[:, 4 * l + oc:4 * l + oc + 1])
                        for oc in range(4):
                            p = ps.tile([128, 512], F32, tag=f"b{4 + oc}")
                            for ic in range(4):
                                nc.tensor.matmul(
                                    p[:],
                                    w2s[:, 512 * ic + 128 * oc:
                                        512 * ic + 128 * oc + 128],
                                    u1c[ic][:],
                                    start=(ic == 0), stop=(ic == 3))
                            nc.vector.tensor_copy(mt[oc][:, ncol:ncol + 512],
                                                  p[:])
                            col = 4 * j + oc
                            sq = gp.tile([128, 512], F32, tag="sq", bufs=2)
                            nc.scalar.activation(sq[:], p[:], AF.Square)
                            nc.vector.reduce_sum(st16s[:, col:col + 1],
                                                 mt[oc][:, ncol:ncol + 512],
                                                 axis=AX.X)
                            nc.vector.reduce_sum(st16q[:, col:col + 1], sq[:],
                                                 axis=AX.X)

                    # assemble + AllReduce BN stats
                    stat = gp.tile([128, 8], F32, tag="stat")
                    t4a = gp.tile([128, 4], F32, tag="t4a")
                    t4b = gp.tile([128, 4], F32, tag="t4b")
                    nc.vector.tensor_tensor(out=t4a[:], in0=st16s[:, 0:4],
                                            in1=st16s[:, 4:8], op=ALU.add)
                    nc.vector.tensor_tensor(out=t4b[:], in0=st16s[:, 8:12],
                                            in1=st16s[:, 12:16], op=ALU.add)
                    nc.vector.tensor_tensor(out=stat[:, 0:4], in0=t4a[:],
                                            in1=t4b[:], op=ALU.add)
                    t4c = gp.tile([128, 4], F32, tag="t4c")
                    t4d = gp.tile([128, 4], F32, tag="t4d")
                    nc.vector.tensor_tensor(out=t4c[:], in0=st16q[:, 0:4],
                                            in1=st16q[:, 4:8], op=ALU.add)
                    nc.vector.tensor_tensor(out=t4d[:], in0=st16q[:, 8:12],
                                            in1=st16q[:, 12:16], op=ALU.add)
                    nc.vector.tensor_tensor(out=stat[:, 4:8], in0=t4c[:],
                                            in1=t4d[:], op=ALU.add)
                    nc.sync.dma_start(bn_loc[l][:, :], stat[:])
                    nc.gpsimd.collective_compute(
                        "AllReduce", ALU.add, ins=[bn_loc[l][:, :].opt()],
                        outs=[bn_glob[l][:, :].opt()], replica_groups=RG)
                    ga = gp.tile([128, 8], F32, tag="ga")
                    nc.sync.dma_start(ga[:], bn_glob[l][:, :])

                    # BN coefficients, vectorized over the 4 feature blocks
                    mu4 = gp.tile([128, 4], F32, tag="mu4")
                    nc.vector.tensor_scalar(out=mu4[:], in0=ga[:, 0:4],
                                            scalar1=1.0 / N, scalar2=None,
                                            op0=ALU.mult)
                    ex24 = gp.tile([128, 4], F32, tag="ex24")
                    nc.vector.tensor_scalar(out=ex24[:], in0=ga[:, 4:8],
                                            scalar1=1.0 / N, scalar2=None,
                                            op0=ALU.mult)
                    mu2 = gp.tile([128, 4], F32, tag="mu2")
                    nc.vector.tensor_tensor(out=mu2[:], in0=mu4[:], in1=mu4[:],
                                            op=ALU.mult)
                    var4 = gp.tile([128, 4], F32, tag="var4")
                    nc.vector.tensor_tensor(out=var4[:], in0=ex24[:],
                                            in1=mu2[:], op=ALU.subtract)
                    vare = gp.tile([128, 4], F32, tag="vare")
                    nc.vector.tensor_scalar(out=vare[:], in0=var4[:],
                                            scalar1=BN_EPS, scalar2=None,
                                            op0=ALU.add)
                    std4 = gp.tile([128, 4], F32, tag="std4")
                    nc.scalar.activation(std4[:], vare[:], AF.Sqrt)
                    inv4 = gp.tile([128, 4], F32, tag="inv4")
                    nc.vector.reciprocal(inv4[:], std4[:])
                    sv4 = gp.tile([128, 4], F32, tag="sv4")
                    nc.vector.tensor_tensor(out=sv4[:], in0=inv4[:],
                                            in1=bngw[:, 4 * l:4 * l + 4],
                                            op=ALU.mult)
                    mst = gp.tile([128, 4], F32, tag="mst")
                    nc.vector.tensor_tensor(out=mst[:], in0=mu4[:], in1=sv4[:],
                                            op=ALU.mult)
                    tv4 = gp.tile([128, 4], F32, tag="tv4")
                    nc.vector.tensor_tensor(out=tv4[:],
                                            in0=bnbw[:, 4 * l:4 * l + 4],
                                            in1=mst[:], op=ALU.subtract)

                    # BN apply + residual, j-outer so transposes start early
                    for j in range(4):
                        ncol = 512 * j
                        for fc in range(4):
                            rt = gp.tile([128, 512], F32R, tag=f"rt{fc}",
                                         bufs=2, name=f"rt{fc}")
                            nc.scalar.activation(
                                rt[:], mt[fc][:, ncol:ncol + 512], AF.Relu,
                                bias=tv4[:, fc:fc + 1], scale=sv4[:, fc:fc + 1])
                            nc.vector.tensor_tensor(
                                out=hT[fc][:, ncol:ncol + 512], in0=rt[:],
                                in1=hT[fc][:, ncol:ncol + 512], op=ALU.add)
                        if l < L - 1:
                            for nb in range(4 * j, 4 * j + 4):
                                hn2 = gp.tile([128, 2 * 512], F8, tag="hn2",
                                              bufs=2)
                                for fc in range(4):
                                    pt = ps.tile([128, 128], F32, tag=f"b{fc}")
                                    nc.tensor.transpose(
                                        pt[:],
                                        hT[fc][:, 128 * nb:128 * nb + 128]
                                        .bitcast(F32),
                                        ident[:])
                                    nc.vector.tensor_copy(
                                        hn2[:, 128 * fc:128 * fc + 128], pt[:])
                                    h32 = gp.tile([128, 128], F32, tag="h32",
                                                  bufs=2)
                                    nc.vector.tensor_copy(
                                        h32[:],
                                        hn2[:, 128 * fc:128 * fc + 128])
                                    nc.vector.tensor_tensor(
                                        out=hn2[:, 512 + 128 * fc:
                                                512 + 128 * fc + 128],
                                        in0=pt[:], in1=h32[:],
                                        op=ALU.subtract)
                                sl = sA[l + 1] if nb < 8 else sB[l + 1]
                                nc.sync.dma_start(
                                    sl[(nb % 8) // 2, :, nb % 2, :], hn2[:])
                            if j == 1:
                                nc.gpsimd.collective_compute(
                                    "AllGather", ALU.bypass,
                                    ins=[sA[l + 1][:, :, :, :].opt()],
                                    outs=[tA[l + 1][:, :, :, :].opt()],
                                    replica_groups=RG)
                            if j == 3:
                                nc.gpsimd.collective_compute(
                                    "AllGather", ALU.bypass,
                                    ins=[sB[l + 1][:, :, :, :].opt()],
                                    outs=[tB[l + 1][:, :, :, :].opt()],
                                    replica_groups=RG)

            # ---------------- final MLP + heads + pairwise ----------------
            with tc.tile_pool(name="fin", bufs=1) as fp:
                mwa = [fp.tile([128, DFF], F32R, tag=f"mw1_{ic}", name=f"mwa{ic}")
                       for ic in range(4)]
                for ic in range(4):
                    nc.sync.dma_start(mwa[ic][:],
                                      mw1[128 * ic:128 * ic + 128, :])
                mwb = [fp.tile([128, DFF], F32R, tag=f"mw2_{ic}", name=f"mwb{ic}")
                       for ic in range(8)]
                for ic in range(8):
                    nc.sync.dma_start(mwb[ic][:],
                                      mw2[128 * ic:128 * ic + 128, :])
                mwc = [fp.tile([128, ZI], F32R, tag=f"mw3_{ic}", name=f"mwc{ic}")
                       for ic in range(8)]
                for ic in range(8):
                    nc.sync.dma_start(mwc[ic][:],
                                      mw3[128 * ic:128 * ic + 128, :])
                hw1s = fp.tile([ZI, 2 * ZI], F32R, tag="hw1s")
                nc.sync.dma_start(hw1s[:], hw1b[0:ZI, :])
                hw1s2 = fp.tile([ZI, 2 * ZI], F32R, tag="hw1s2")
                nc.sync.dma_start(hw1s2[:], hw1b[ZI:2 * ZI, :])
                hw2s = fp.tile([128, 2 * ZI], F32R, tag="hw2s")
                nc.sync.dma_start(hw2s[:], hw2b[0:128, :])
                hw2s2 = fp.tile([128, 2 * ZI], F32R, tag="hw2s2")
                nc.sync.dma_start(hw2s2[:], hw2b[128:256, :])
                hw1p = [hw1s, hw1s2]
                hw2p = [hw2s, hw2s2]

                # stage Z: final MLP for all local graphs
                z3g = [fp.tile([ZI, 512], F32R, tag=f"z3_{g}", name=f"z3_{g}")
                       for g in range(GL)]
                for g in range(GL):
                    gcol = 512 * g
                    z1 = [fp.tile([128, 512], F32R, tag=f"z1_{oc}", bufs=1,
                                  name=f"z1_{oc}") for oc in range(8)]
                    for oc in range(8):
                        p = ps.tile([128, 512], F32, tag=f"b{oc}")
                        for ic in range(4):
                            nc.tensor.matmul(
                                p[:],
                                mwa[ic][:, 128 * oc:128 * oc + 128],
                                hT[ic][:, gcol:gcol + 512],
                                start=(ic == 0), stop=(ic == 3))
                        nc.scalar.activation(z1[oc][:], p[:], AF.Relu,
                                             bias=mb1w[:, oc:oc + 1])
                    z2 = [fp.tile([128, 512], F32R, tag=f"z2_{oc}", bufs=1,
                                  name=f"z2_{oc}") for oc in range(8)]
                    for oc in range(8):
                        p = ps.tile([128, 512], F32, tag=f"b{oc}")
                        for ic in range(8):
                            nc.tensor.matmul(
                                p[:],
                                mwb[ic][:, 128 * oc:128 * oc + 128],
                                z1[ic][:],
                                start=(ic == 0), stop=(ic == 7))
                        nc.scalar.activation(z2[oc][:], p[:], AF.Relu,
                                             bias=mb2w[:, oc:oc + 1])
                    pz = ps.tile([ZI, 512], F32, tag="b0")
                    for ic in range(8):
                        nc.tensor.matmul(pz[:], mwc[ic][:, 0:ZI], z2[ic][:],
                                         start=(ic == 0), stop=(ic == 7))
                    nc.vector.tensor_tensor(
                        out=z3g[g][:], in0=pz[:],
                        in1=mb3w[:, 0:1].to_broadcast([ZI, 512])[:],
                        op=ALU.add)

                # stage H: heads in pairs (2 heads per 128-partition op)
                for g in range(GL):
                    for kp in range(2):
                        p1 = ps.tile([128, 512], F32, tag="b1")
                        nc.tensor.matmul(p1[:], hw1p[kp][:], z3g[g][:],
                                         start=True, stop=True)
                        h1 = fp.tile([128, 512], F32R, tag="h1", bufs=2)
                        nc.scalar.activation(h1[:], p1[:], AF.Relu,
                                             bias=hb1w[:, kp:kp + 1])
                        p2 = ps.tile([128, 512], F32, tag="b2")
                        nc.tensor.matmul(p2[:], hw2p[kp][:], h1[:],
                                         start=True, stop=True)
                        hkt = fp.tile([128, 512], F32, tag="hkt", bufs=2)
                        nc.vector.tensor_tensor(
                            out=hkt[:], in0=p2[:],
                            in1=hb2w[:, kp:kp + 1].to_broadcast([128, 512])[:],
                            op=ALU.add)
                        hm2 = fp.tile([128, 512], F32, tag="hm2", bufs=2)
                        nc.vector.tensor_scalar(out=hm2[:], in0=hkt[:],
                                                scalar1=-2.0, scalar2=None,
                                                op0=ALU.mult)
                        sqt = fp.tile([128, 512], F32, tag="sqt", bufs=2)
                        nc.vector.tensor_tensor(out=sqt[:], in0=hkt[:],
                                                in1=hkt[:], op=ALU.mult)
                        rsb = [None, None]
                        for h in range(2):
                            prh = ps.tile([1, 512], F32, tag="b0")
                            nc.tensor.matmul(prh[:], selm[:, h:h + 1], sqt[:],
                                             start=True, stop=True)
                            rsb[h] = fp.tile([1, 512], F32, tag=f"rsb{h}",
                                             bufs=2, name=f"rsb{h}")
                            nc.vector.tensor_copy(rsb[h][:], prh[:])
                        d2t = [None] * 8
                        for h in range(2):
                            for mb in range(4):
                                pd = ps.tile([128, 512], F32, tag=f"b{4 + mb}")
                                nc.tensor.matmul(
                                    pd[:],
                                    hm2[64 * h:64 * h + 64,
                                        128 * mb:128 * mb + 128],
                                    hkt[64 * h:64 * h + 64, :],
                                    start=True, stop=False)
                                nc.tensor.matmul(pd[:], onf[:, 0:128],
                                                 rsb[h][:], start=False,
                                                 stop=False,
                                                 skip_group_check=True)
                                nc.tensor.matmul(
                                    pd[:], rsb[h][:, 128 * mb:128 * mb + 128],
                                    onf[:], start=False, stop=True,
                                    skip_group_check=True)
                                i8 = 4 * h + mb
                                d2t[i8] = fp.tile([128, 512], F32, tag=f"d2_{i8}",
                                                  name=f"d2_{i8}")
                                nc.vector.tensor_scalar(out=d2t[i8][:],
                                                        in0=pd[:],
                                                        scalar1=1e-12,
                                                        scalar2=None,
                                                        op0=ALU.max)
                        lnt = [None] * 8
                        for i8 in range(8):
                            lnt[i8] = fp.tile([128, 512], F32, tag=f"ln_{i8}",
                                              name=f"ln_{i8}")
                            nc.scalar.activation(lnt[i8][:], d2t[i8][:], AF.Ln)
                        for i8 in range(8):
                            h, mb = i8 // 4, i8 % 4
                            qt = fp.tile([128, 512], F32, tag="qt", bufs=2)
                            nc.scalar.activation(qt[:], lnt[i8][:], AF.Sigmoid,
                                                 bias=nla[:, 0:1],
                                                 scale=-UMAP_B)
                            row = ((g * K + 2 * kp + h) * 4 + mb) * 128
                            nc.sync.dma_start(qout[row:row + 128, :], qt[:])
    nc.compile()
    return nc


def _host_prep(inputs):
    x = np.asarray(inputs["x"], np.float32)
    edge_index = np.asarray(inputs["edge_index"], np.int64)
    src, dst = edge_index[0], edge_index[1]
    hw1 = np.asarray(inputs["head_w1"], np.float32)
    hw2 = np.asarray(inputs["head_w2"], np.float32)
    hb1 = np.asarray(inputs["head_b1"], np.float32)
    hb2 = np.asarray(inputs["head_b2"], np.float32)

    hw1b = np.zeros((2 * ZI, 2 * ZI), np.float32)
    hw2b = np.zeros((4 * ZI, 2 * ZI), np.float32)
    hb1b = np.zeros((128, 2), np.float32)
    hb2b = np.zeros((128, 2), np.float32)
    for kp in range(2):
        hw1b[ZI * kp:ZI * kp + ZI, 0:ZI] = hw1[2 * kp]
        hw1b[ZI * kp:ZI * kp + ZI, ZI:2 * ZI] = hw1[2 * kp + 1]
        hw2b[128 * kp:128 * kp + ZI, 0:ZI] = hw2[2 * kp]
        hw2b[128 * kp + ZI:128 * kp + 128, ZI:2 * ZI] = hw2[2 * kp + 1]
        hb1b[0:ZI, kp] = hb1[2 * kp]
        hb1b[ZI:128, kp] = hb1[2 * kp + 1]
        hb2b[0:ZI, kp] = hb2[2 * kp]
        hb2b[ZI:128, kp] = hb2[2 * kp + 1]
    selm = np.zeros((128, 2), np.float32)
    selm[0:ZI, 0] = 1.0
    selm[ZI:128, 1] = 1.0

    shared = {
        "embw": np.ascontiguousarray(np.vstack(
            [np.asarray(inputs["emb_w"], np.float32),
             np.asarray(inputs["emb_b"], np.float32)[None, :]])),
        "gw1": np.ascontiguousarray(
            np.asarray(inputs["gin_w1"], np.float32).reshape(L * D, D)),
        "gw2": np.ascontiguousarray(
            np.asarray(inputs["gin_w2"], np.float32).reshape(L * D, D)),
        "mw1": np.ascontiguousarray(np.asarray(inputs["mlp_w1"], np.float32)),
        "mw2": np.ascontiguousarray(np.asarray(inputs["mlp_w2"], np.float32)),
        "mw3": np.ascontiguousarray(np.asarray(inputs["mlp_w3"], np.float32)),
        "hw1b": hw1b, "hw2b": hw2b,
        "hb1b_d": hb1b, "hb2b_d": hb2b, "selm_d": selm,
        "gb1_d": np.ascontiguousarray(
            np.asarray(inputs["gin_b1"], np.float32)
            .reshape(L, 4, 128).transpose(2, 0, 1).reshape(128, 16)),
        "bng_d": np.ascontiguousarray(
            np.asarray(inputs["bn_g"], np.float32)
            .reshape(L, 4, 128).transpose(2, 0, 1).reshape(128, 16)),
        "bnb_d": np.ascontiguousarray(
            np.asarray(inputs["bn_b"], np.float32)
            .reshape(L, 4, 128).transpose(2, 0, 1).reshape(128, 16)),
        "mb1_d": np.ascontiguousarray(
            np.asarray(inputs["mlp_b1"], np.float32).reshape(8, 128).T),
        "mb2_d": np.ascontiguousarray(
            np.asarray(inputs["mlp_b2"], np.float32).reshape(8, 128).T),
        "mb3_d": np.ascontiguousarray(
            np.asarray(inputs["mlp_b3"], np.float32)[:, None]),
    }

    # layer-0 exact aggregation operands
    emb_w = np.asarray(inputs["emb_w"], np.float32)
    emb_b = np.asarray(inputs["emb_b"], np.float32)
    xhi = x.astype(ml_dtypes.float8_e4m3)
    xlo = (x - xhi.astype(np.float32)).astype(ml_dtypes.float8_e4m3)
    xa = np.zeros((N, 32), ml_dtypes.float8_e4m3)
    xa[:, 0:9] = xhi
    xa[:, 10:19] = xlo
    xa[:, 20] = np.float32(1.0)
    shared["xg8"] = np.ascontiguousarray(
        xa.reshape(KB2, 2, 128, 32).transpose(2, 0, 1, 3))
    ewa = np.zeros((32, D), np.float32)
    ewa[0:9] = emb_w
    ewa[10:19] = emb_w
    ewa[20] = emb_b
    shared["embwa"] = ewa

    in_maps = []
    ones_row = np.ones((1, NL), np.float32)
    for c in range(NCORES):
        lo = NL * c
        mask = (dst >= lo) & (dst < lo + NL)
        flat = src[mask] * NL + (dst[mask] - lo)
        a = np.bincount(flat, minlength=N * NL).astype(np.float32)
        # paired layout for DoubleRow: [p, kb2, two, dst]
        a = a.reshape(KB2, 2, 128, NL).transpose(2, 0, 1, 3)
        m = dict(shared)
        m["acm8"] = np.ascontiguousarray(a).astype(ml_dtypes.float8_e4m3)
        m["xt"] = np.ascontiguousarray(
            np.vstack([x[lo:lo + NL].T, ones_row]))
        in_maps.append(m)
    return in_maps


def kernel(**inputs) -> np.ndarray:
    global _NC_CACHE
    if _NC_CACHE is None:
        _NC_CACHE = build_nc()
    nc = _NC_CACHE
    in_maps = _host_prep(inputs)
    res = run_bass_kernel_spmd(nc, in_maps, core_ids=list(range(NCORES)))
    out = np.concatenate(
        [np.asarray(res.results[c]["qout"]).reshape(GL, K, NG, NG)
         for c in range(NCORES)], axis=0)
    return out


# revision 22
# speedup vs baseline: 1.5504x; 1.0145x over previous
import sys
sys.path.insert(0, "/opt/trn_rl_repo")
import math
import numpy as np
import ml_dtypes

import concourse.bass as bass
from concourse import bacc, mybir
from concourse.tile import TileContext
from concourse.bass_utils import run_bass_kernel_spmd
from concourse.masks import make_identity

F32 = mybir.dt.float32
F32R = mybir.dt.float32r
F8 = mybir.dt.float8e4
AF = mybir.ActivationFunctionType
ALU = mybir.AluOpType
AX = mybir.AxisListType
DR = mybir.MatmulPerfMode.DoubleRow

N, G, E = 16384, 32, 524288
D, DFF, ZI, K, L = 512, 1024, 64, 4, 4
UMAP_A, UMAP_B = 1.577, 0.8951
BN_EPS = 1e-5
NCORES = 8
NL = N // NCORES      # 2048 local nodes per core
GL = G // NCORES      # 4 local graphs per core
NG = N // G           # 512 nodes per graph
KB2 = N // 256        # 64 paired source blocks (256 src nodes each)
RG = [list(range(NCORES))]

_NC_CACHE = None


def build_nc():
    nc = bacc.Bacc("TRN2", target_bir_lowering=False, debug=False,
                   enable_asserts=True, num_devices=NCORES)

    xt = nc.dram_tensor("xt", (10, NL), F32R, kind="ExternalInput")
    # full-graph x in paired fp8: cols = [x_hi(10) | x_lo(10) | ones(1) | pad]
    xg8 = nc.dram_tensor("xg8", (128, KB2, 2, 32), F8, kind="ExternalInput")
    # emb_w stacked for the hi/lo recombine: rows = [W, W, b, 0...]
    embwa = nc.dram_tensor("embwa", (32, D), F32R, kind="ExternalInput")
    acm8 = nc.dram_tensor("acm8", (128, KB2, 2, NL), F8, kind="ExternalInput")
    embw = nc.dram_tensor("embw", (10, D), F32R, kind="ExternalInput")
    gw1 = nc.dram_tensor("gw1", (L * D, D), F32R, kind="ExternalInput")
    gw2 = nc.dram_tensor("gw2", (L * D, D), F32R, kind="ExternalInput")
    mw1 = nc.dram_tensor("mw1", (D, DFF), F32R, kind="ExternalInput")
    mw2 = nc.dram_tensor("mw2", (DFF, DFF), F32R, kind="ExternalInput")
    mw3 = nc.dram_tensor("mw3", (DFF, ZI), F32R, kind="ExternalInput")
    # head weights batched in pairs: hw1b[64*kp:64*kp+64] = [w1_{2kp}|w1_{2kp+1}]
    hw1b = nc.dram_tensor("hw1b", (2 * ZI, 2 * ZI), F32R, kind="ExternalInput")
    # hw2b[128*kp:...] = blockdiag(w2_{2kp}, w2_{2kp+1})
    hw2b = nc.dram_tensor("hw2b", (2 * 2 * ZI, 2 * ZI), F32R,
                          kind="ExternalInput")
    gb1_d = nc.dram_tensor("gb1_d", (128, 16), F32, kind="ExternalInput")
    bng_d = nc.dram_tensor("bng_d", (128, 16), F32, kind="ExternalInput")
    bnb_d = nc.dram_tensor("bnb_d", (128, 16), F32, kind="ExternalInput")
    mb1_d = nc.dram_tensor("mb1_d", (128, 8), F32, kind="ExternalInput")
    mb2_d = nc.dram_tensor("mb2_d", (128, 8), F32, kind="ExternalInput")
    mb3_d = nc.dram_tensor("mb3_d", (ZI, 1), F32, kind="ExternalInput")
    hb1b_d = nc.dram_tensor("hb1b_d", (128, 2), F32, kind="ExternalInput")
    hb2b_d = nc.dram_tensor("hb2b_d", (128, 2), F32, kind="ExternalInput")
    selm_d = nc.dram_tensor("selm_d", (128, 2), F32, kind="ExternalInput")
    qout = nc.dram_tensor("qout", (GL * K * 4 * 128, NG), F32,
                          kind="ExternalOutput")

    with TileContext(nc) as tc:
        with (
            tc.tile_pool(name="const", bufs=1) as cp,
            tc.tile_pool(name="res", bufs=1) as rp,
            tc.tile_pool(name="ps", bufs=1, space="PSUM") as ps,
            tc.tile_pool(name="dram", bufs=1, space="DRAM") as dp,
        ):
            ident = cp.tile([128, 128], F32, tag="ident")
            make_identity(nc, ident[:])
            nla = cp.tile([128, 1], F32, tag="nla")
            nc.gpsimd.memset(nla[:], -math.log(UMAP_A))
            onf = cp.tile([1, NG], F32, tag="onf")
            nc.gpsimd.memset(onf[:], 1.0)

            gb1w = cp.tile([128, 16], F32, tag="gb1w")
            nc.sync.dma_start(gb1w[:], gb1_d[:, :])
            bngw = cp.tile([128, 16], F32, tag="bngw")
            nc.sync.dma_start(bngw[:], bng_d[:, :])
            bnbw = cp.tile([128, 16], F32, tag="bnbw")
            nc.sync.dma_start(bnbw[:], bnb_d[:, :])
            mb1w = cp.tile([128, 8], F32, tag="mb1w")
            nc.sync.dma_start(mb1w[:], mb1_d[:, :])
            mb2w = cp.tile([128, 8], F32, tag="mb2w")
            nc.sync.dma_start(mb2w[:], mb2_d[:, :])
            mb3w = cp.tile([ZI, 1], F32, tag="mb3w")
            nc.sync.dma_start(mb3w[:], mb3_d[:, :])
            hb1w = cp.tile([128, 2], F32, tag="hb1w")
            nc.sync.dma_start(hb1w[:], hb1b_d[:, :])
            hb2w = cp.tile([128, 2], F32, tag="hb2w")
            nc.sync.dma_start(hb2w[:], hb2b_d[:, :])
            selm = cp.tile([128, 2], F32, tag="selm")
            nc.sync.dma_start(selm[:], selm_d[:, :])

            hT = [rp.tile([128, NL], F32R, tag=f"hT{fc}", name=f"hT{fc}")
                  for fc in range(4)]

            # chunked hi/lo tables: A covers local kb2 0-3, B covers 4-7
            sA = [dp.tile([4, 128, 2, 2 * D], F8, tag=f"sA{i}", name=f"sA{i}")
                  for i in range(L)]
            sB = [dp.tile([4, 128, 2, 2 * D], F8, tag=f"sB{i}", name=f"sB{i}")
                  for i in range(L)]
            tA = [dp.tile([KB2 // 2, 128, 2, 2 * D], F8, tag=f"tA{i}",
                          name=f"tA{i}", addr_space="Shared")
                  for i in range(L)]
            tB = [dp.tile([KB2 // 2, 128, 2, 2 * D], F8, tag=f"tB{i}",
                          name=f"tB{i}", addr_space="Shared")
                  for i in range(L)]
            bn_loc = [dp.tile([128, 8], F32, tag=f"bl{i}", name=f"bl{i}")
                      for i in range(L)]
            bn_glob = [dp.tile([128, 8], F32, tag=f"bg{i}", name=f"bg{i}",
                               addr_space="Shared")
                       for i in range(L)]
            war_l = dp.tile([128, 8], F32, name="war_l")
            war_g = dp.tile([128, 8], F32, name="war_g", addr_space="Shared")

            # ---------------- embedding + collective warmup ----------------
            with tc.tile_pool(name="emb", bufs=1) as ep:
                xt_sb = ep.tile([10, NL], F32R, tag="xt")
                nc.sync.dma_start(xt_sb[:], xt[:, :])
                ew_sb = ep.tile([10, D], F32R, tag="ew")
                nc.sync.dma_start(ew_sb[:], embw[:, :])
                wz = ep.tile([128, 8], F32, tag="wz")
                nc.gpsimd.memset(wz[:], 0.0)
                nc.sync.dma_start(war_l[:, :], wz[:])
                nc.gpsimd.collective_compute(
                    "AllReduce", ALU.add, ins=[war_l[:, :].opt()],
                    outs=[war_g[:, :].opt()], replica_groups=RG)
                for fc in range(4):
                    for j in range(4):
                        p = ps.tile([128, 512], F32, tag=f"b{4 + fc}")
                        nc.tensor.matmul(p[:], ew_sb[:, 128 * fc:128 * fc + 128],
                                         xt_sb[:, 512 * j:512 * j + 512],
                                         start=True, stop=True)
                        nc.vector.tensor_copy(hT[fc][:, 512 * j:512 * j + 512],
                                              p[:])

            # ---------------- GIN layers ----------------
            with tc.tile_pool(name="gin", bufs=1) as gp:
                for l in range(L):
                    w1s = gp.tile([128, 2048], F32R, tag="w1")
                    w2s = gp.tile([128, 2048], F32R, tag="w2")
                    for ic in range(4):
                        r0 = 512 * l + 128 * ic
                        nc.sync.dma_start(w1s[:, 512 * ic:512 * ic + 512],
                                          gw1[r0:r0 + 128, :])
                        nc.sync.dma_start(w2s[:, 512 * ic:512 * ic + 512],
                                          gw2[r0:r0 + 128, :])

                    mt = [gp.tile([128, NL], F32R, tag=f"mt{fc}",
                                  name=f"mt{fc}_{l}") for fc in range(4)]

                    if l == 0:
                        # layer-0 aggregation, exact: agg0 = (A.T xa) @ Wa
                        xg_sb = gp.tile([128, KB2, 2, 32], F8, tag="xg")
                        nc.sync.dma_start(xg_sb[:], xg8[:, :, :, :])
                        ewa = gp.tile([32, D], F32R, tag="ewa")
                        nc.sync.dma_start(ewa[:], embwa[:, :])
                        px = [ps.tile([32, 512], F32, tag=f"b{i}",
                                      name=f"px{i}") for i in range(4)]
                        for half in range(2):
                            for kb2 in range(KB2):
                                a8 = gp.tile([128, 2, 1024], F8, tag="a8",
                                             bufs=4)
                                eng = nc.sync if kb2 % 2 == 0 else nc.scalar
                                eng.dma_start(
                                    a8[:],
                                    acm8[:, kb2, :,
                                         1024 * half:1024 * half + 1024])
                                lhs = xg_sb[:, kb2, :, :]
                                for dc in range(2):
                                    nc.tensor.matmul(
                                        px[2 * half + dc][:], lhs,
                                        a8[:, :, 512 * dc:512 * dc + 512],
                                        start=(kb2 == 0), stop=(kb2 == KB2 - 1),
                                        perf_mode=DR)
                        atxt = gp.tile([32, NL], F32R, tag="atxt")
                        for dt in range(4):
                            nc.vector.tensor_copy(
                                atxt[:, 512 * dt:512 * dt + 512], px[dt][:])
                        for d in range(4):
                            for jc in range(4):
                                pr0 = ps.tile([128, 512], F32, tag=f"b{4 + jc}")
                                nc.tensor.matmul(
                                    pr0[:], ewa[:, 128 * d:128 * d + 128],
                                    atxt[:, 512 * jc:512 * jc + 512],
                                    start=True, stop=True)
                                nc.vector.tensor_tensor(
                                    out=mt[d][:, 512 * jc:512 * jc + 512],
                                    in0=pr0[:],
                                    in1=hT[d][:, 512 * jc:512 * jc + 512],
                                    op=ALU.add)
                    else:
                        # aggregation: aggT = (hi + lo).T @ A8 (DoubleRow) + hT
                        # consume chunk-A table rows first, then chunk-B
                        order = ([("A", r) for r in range(KB2 // 2)]
                                 + [("B", r) for r in range(KB2 // 2)])
                        for half in range(2):
                            pb = [ps.tile([128, 512], F32, tag=f"b{i}",
                                          name=f"pb{i}") for i in range(8)]
                            for ii, (ab, r) in enumerate(order):
                                gkb2 = 8 * (r // 4) + (r % 4) + (4 if ab == "B"
                                                                 else 0)
                                a8 = gp.tile([128, 2, 1024], F8, tag="a8",
                                             bufs=4)
                                nc.sync.dma_start(
                                    a8[:],
                                    acm8[:, gkb2, :,
                                         1024 * half:1024 * half + 1024])
                                hk8 = gp.tile([128, 2, 1024], F8, tag="hk8",
                                              bufs=4)
                                tab = tA[l] if ab == "A" else tB[l]
                                nc.scalar.dma_start(hk8[:], tab[r, :, :, :])
                                for d in range(4):
                                    for hilo in range(2):
                                        lhs = hk8[:, :,
                                                  512 * hilo + 128 * d:
                                                  512 * hilo + 128 * d + 128]
                                        for dc in range(2):
                                            nc.tensor.matmul(
                                                pb[2 * d + dc][:], lhs,
                                                a8[:, :,
                                                   512 * dc:512 * dc + 512],
                                                start=(ii == 0 and hilo == 0),
                                                stop=(ii == KB2 - 1
                                                      and hilo == 1),
                                                perf_mode=DR)
                            for d in range(4):
                                for dc in range(2):
                                    col = 1024 * half + 512 * dc
                                    nc.vector.tensor_tensor(
                                        out=mt[d][:, col:col + 512],
                                        in0=pb[2 * d + dc][:],
                                        in1=hT[d][:, col:col + 512],
                                        op=ALU.add)

                    # GIN MLP: u1 = relu(m@w1+b1); u2 = u1@w2 (into mt)
                    # fold BN partial stats in as u2 chunks are produced
                    st16s = gp.tile([128, 16], F32, tag="st16s")
                    st16q = gp.tile([128, 16], F32, tag="st16q")
                    for j in range(4):
                        ncol = 512 * j
                        u1c = [gp.tile([128, 512], F32R, tag=f"u1_{oc}", bufs=2,
                                       name=f"u1c{oc}") for oc in range(4)]
                        for oc in range(4):
                            p = ps.tile([128, 512], F32, tag=f"b{oc}")
                            for ic in range(4):
                                nc.tensor.matmul(
                                    p[:],
                                    w1s[:, 512 * ic + 128 * oc:
                                        512 * ic + 128 * oc + 128],
                                    mt[ic][:, ncol:ncol + 512],
                                    start=(ic == 0), stop=(ic == 3))
                            nc.scalar.activation(
                                u1c[oc][:], p[:], AF.Relu,
                                bias=gb1w[:, 4 * l + oc:4 * l + oc + 1])
                        for oc in range(4):
                            p = ps.tile([128, 512], F32, tag=f"b{4 + oc}")
                            for ic in range(4):
                                nc.tensor.matmul(
                                    p[:],
                                    w2s[:, 512 * ic + 128 * oc:
                                        512 * ic + 128 * oc + 128],
                                    u1c[ic][:],
                                    start=(ic == 0), stop=(ic == 3))
                            nc.vector.tensor_copy(mt[oc][:, ncol:ncol + 512],
                                                  p[:])
                            col = 4 * j + oc
                            sq = gp.tile([128, 512], F32, tag="sq", bufs=2)
                            nc.scalar.activation(sq[:], p[:], AF.Square)
                            nc.vector.reduce_sum(st16s[:, col:col + 1],
                                                 mt[oc][:, ncol:ncol + 512],
                                                 axis=AX.X)
                            nc.vector.reduce_sum(st16q[:, col:col + 1], sq[:],
                                                 axis=AX.X)

                    # assemble + AllReduce BN stats
                    stat = gp.tile([128, 8], F32, tag="stat")
                    t4a = gp.tile([128, 4], F32, tag="t4a")
                    t4b = gp.tile([128, 4], F32, tag="t4b")
                    nc.vector.tensor_tensor(out=t4a[:], in0=st16s[:, 0:4],
                                            in1=st16s[:, 4:8], op=ALU.add)
                    nc.vector.tensor_tensor(out=t4b[:], in0=st16s[:, 8:12],
                                            in1=st16s[:, 12:16], op=ALU.add)
                    nc.vector.tensor_tensor(out=stat[:, 0:4], in0=t4a[:],
                                            in1=t4b[:], op=ALU.add)
                    t4c = gp.tile([128, 4], F32, tag="t4c")
                    t4d = gp.tile([128, 4], F32, tag="t4d")
                    nc.vector.tensor_tensor(out=t4c[:], in0=st16q[:, 0:4],
                                            in1=st16q[:, 4:8], op=ALU.add)
                    nc.vector.tensor_tensor(out=t4d[:], in0=st16q[:, 8:12],
                                            in1=st16q[:, 12:16], op=ALU.add)
                    nc.vector.tensor_tensor(out=stat[:, 4:8], in0=t4c[:],
                                            in1=t4d[:], op=ALU.add)
                    nc.sync.dma_start(bn_loc[l][:, :], stat[:])
                    nc.gpsimd.collective_compute(
                        "AllReduce", ALU.add, ins=[bn_loc[l][:, :].opt()],
                        outs=[bn_glob[l][:, :].opt()], replica_groups=RG)
                    ga = gp.tile([128, 8], F32, tag="ga")
                    nc.sync.dma_start(ga[:], bn_glob[l][:, :])

                    # BN coefficients, vectorized over the 4 feature blocks
                    mu4 = gp.tile([128, 4], F32, tag="mu4")
                    nc.vector.tensor_scalar(out=mu4[:], in0=ga[:, 0:4],
                                            scalar1=1.0 / N, scalar2=None,
                                            op0=ALU.mult)
                    ex24 = gp.tile([128, 4], F32, tag="ex24")
                    nc.vector.tensor_scalar(out=ex24[:], in0=ga[:, 4:8],
                                            scalar1=1.0 / N, scalar2=None,
                                            op0=ALU.mult)
                    mu2 = gp.tile([128, 4], F32, tag="mu2")
                    nc.vector.tensor_tensor(out=mu2[:], in0=mu4[:], in1=mu4[:],
                                            op=ALU.mult)
                    var4 = gp.tile([128, 4], F32, tag="var4")
                    nc.vector.tensor_tensor(out=var4[:], in0=ex24[:],
                                            in1=mu2[:], op=ALU.subtract)
                    vare = gp.tile([128, 4], F32, tag="vare")
                    nc.vector.tensor_scalar(out=vare[:], in0=var4[:],
                                            scalar1=BN_EPS, scalar2=None,
                                            op0=ALU.add)
                    std4 = gp.tile([128, 4], F32, tag="std4")
                    nc.scalar.activation(std4[:], vare[:], AF.Sqrt)
                    inv4 = gp.tile([128, 4], F32, tag="inv4")
                    nc.vector.reciprocal(inv4[:], std4[:])
                    sv4 = gp.tile([128, 4], F32, tag="sv4")
                    nc.vector.tensor_tensor(out=sv4[:], in0=inv4[:],
                                            in1=bngw[:, 4 * l:4 * l + 4],
                                            op=ALU.mult)
                    mst = gp.tile([128, 4], F32, tag="mst")
                    nc.vector.tensor_tensor(out=mst[:], in0=mu4[:], in1=sv4[:],
                                            op=ALU.mult)
                    tv4 = gp.tile([128, 4], F32, tag="tv4")
                    nc.vector.tensor_tensor(out=tv4[:],
                                            in0=bnbw[:, 4 * l:4 * l + 4],
                                            in1=mst[:], op=ALU.subtract)

                    # BN apply + residual, j-outer so transposes start early
                    for j in range(4):
                        ncol = 512 * j
                        for fc in range(4):
                            rt = gp.tile([128, 512], F32R, tag=f"rt{fc}",
                                         bufs=2, name=f"rt{fc}")
                            nc.scalar.activation(
                                rt[:], mt[fc][:, ncol:ncol + 512], AF.Relu,
                                bias=tv4[:, fc:fc + 1], scale=sv4[:, fc:fc + 1])
                            nc.vector.tensor_tensor(
                                out=hT[fc][:, ncol:ncol + 512], in0=rt[:],
                                in1=hT[fc][:, ncol:ncol + 512], op=ALU.add)
                        if l < L - 1:
                            for nb in range(4 * j, 4 * j + 4):
                                hn2 = gp.tile([128, 2 * 512], F8, tag="hn2",
                                              bufs=2)
                                for fc in range(4):
                                    pt = ps.tile([128, 128], F32, tag=f"b{fc}")
                                    nc.tensor.transpose(
                                        pt[:],
                                        hT[fc][:, 128 * nb:128 * nb + 128]
                                        .bitcast(F32),
                                        ident[:])
                                    nc.vector.tensor_copy(
                                        hn2[:, 128 * fc:128 * fc + 128], pt[:])
                                    h32 = gp.tile([128, 128], F32, tag="h32",
                                                  bufs=2)
                                    nc.vector.tensor_copy(
                                        h32[:],
                                        hn2[:, 128 * fc:128 * fc + 128])
                                    nc.vector.tensor_tensor(
                                        out=hn2[:, 512 + 128 * fc:
                                                512 + 128 * fc + 128],
                                        in0=pt[:], in1=h32[:],
                                        op=ALU.subtract)
                                sl = sA[l + 1] if nb < 8 else sB[l + 1]
                                nc.sync.dma_start(
                                    sl[(nb % 8) // 2, :, nb % 2, :], hn2[:])
                            if j == 1:
                                nc.gpsimd.collective_compute(
                                    "AllGather", ALU.bypass,
                                    ins=[sA[l + 1][:, :, :, :].opt()],
                                    outs=[tA[l + 1][:, :, :, :].opt()],
                                    replica_groups=RG)
                            if j == 3:
                                nc.gpsimd.collective_compute(
                                    "AllGather", ALU.bypass,
                                    ins=[sB[l + 1][:, :, :, :].opt()],
                                    outs=[tB[l + 1][:, :, :, :].opt()],
                                    replica_groups=RG)

            # ---------------- final MLP + heads + pairwise ----------------
            z3g = [rp.tile([ZI, 512], F32R, tag=f"z3_{g}", name=f"z3_{g}")
                   for g in range(GL)]
            with tc.tile_pool(name="finz", bufs=1) as fz:
                mwa = [fz.tile([128, DFF], F32R, tag=f"mw1_{ic}", name=f"mwa{ic}")
                       for ic in range(4)]
                for ic in range(4):
                    nc.sync.dma_start(mwa[ic][:],
                                      mw1[128 * ic:128 * ic + 128, :])
                mwb = [fz.tile([128, DFF], F32R, tag=f"mw2_{ic}", name=f"mwb{ic}")
                       for ic in range(8)]
                for ic in range(8):
                    nc.sync.dma_start(mwb[ic][:],
                                      mw2[128 * ic:128 * ic + 128, :])
                mwc = [fz.tile([128, ZI], F32R, tag=f"mw3_{ic}", name=f"mwc{ic}")
                       for ic in range(8)]
                for ic in range(8):
                    nc.sync.dma_start(mwc[ic][:],
                                      mw3[128 * ic:128 * ic + 128, :])
                for g in range(GL):
                    gcol = 512 * g
                    z1 = [fz.tile([128, 512], F32R, tag=f"z1_{oc}", bufs=2,
                                  name=f"z1_{oc}") for oc in range(8)]
                    for oc in range(8):
                        p = ps.tile([128, 512], F32, tag=f"b{oc}")
                        for ic in range(4):
                            nc.tensor.matmul(
                                p[:],
                                mwa[ic][:, 128 * oc:128 * oc + 128],
                                hT[ic][:, gcol:gcol + 512],
                                start=(ic == 0), stop=(ic == 3))
                        nc.scalar.activation(z1[oc][:], p[:], AF.Relu,
                                             bias=mb1w[:, oc:oc + 1])
                    z2 = [fz.tile([128, 512], F32R, tag=f"z2_{oc}", bufs=2,
                                  name=f"z2_{oc}") for oc in range(8)]
                    for oc in range(8):
                        p = ps.tile([128, 512], F32, tag=f"b{oc}")
                        for ic in range(8):
                            nc.tensor.matmul(
                                p[:],
                                mwb[ic][:, 128 * oc:128 * oc + 128],
                                z1[ic][:],
                                start=(ic == 0), stop=(ic == 7))
                        nc.scalar.activation(z2[oc][:], p[:], AF.Relu,
                                             bias=mb2w[:, oc:oc + 1])
                    pz = ps.tile([ZI, 512], F32, tag="b0")
                    for ic in range(8):
                        nc.tensor.matmul(pz[:], mwc[ic][:, 0:ZI], z2[ic][:],
                                         start=(ic == 0), stop=(ic == 7))
                    nc.vector.tensor_tensor(
                        out=z3g[g][:], in0=pz[:],
                        in1=mb3w[:, 0:1].to_broadcast([ZI, 512])[:],
                        op=ALU.add)

            # heads stage: 8 chains (g, kp) advanced stage-by-stage
            with tc.tile_pool(name="finh", bufs=1) as fh:
                hw1s = fh.tile([ZI, 2 * ZI], F32R, tag="hw1s")
                nc.sync.dma_start(hw1s[:], hw1b[0:ZI, :])
                hw1s2 = fh.tile([ZI, 2 * ZI], F32R, tag="hw1s2")
                nc.sync.dma_start(hw1s2[:], hw1b[ZI:2 * ZI, :])
                hw2s = fh.tile([128, 2 * ZI], F32R, tag="hw2s")
                nc.sync.dma_start(hw2s[:], hw2b[0:128, :])
                hw2s2 = fh.tile([128, 2 * ZI], F32R, tag="hw2s2")
                nc.sync.dma_start(hw2s2[:], hw2b[128:256, :])
                hw1p = [hw1s, hw1s2]
                hw2p = [hw2s, hw2s2]
                CI = [(g, kp) for kp in range(2) for g in range(GL)]
                h1 = [fh.tile([128, 512], F32R, tag=f"h1_{ci}", name=f"h1_{ci}")
                      for ci in range(8)]
                hkt = [fh.tile([128, 512], F32, tag=f"hkt_{ci}",
                               name=f"hkt_{ci}") for ci in range(8)]
                hm2 = [fh.tile([128, 512], F32, tag=f"hm2_{ci}",
                               name=f"hm2_{ci}") for ci in range(8)]
                sqt = [fh.tile([128, 512], F32, tag=f"sqt_{ci}",
                               name=f"sqt_{ci}") for ci in range(8)]
                for ci, (g, kp) in enumerate(CI):
                    p1 = ps.tile([128, 512], F32, tag=f"b{ci}")
                    nc.tensor.matmul(p1[:], hw1p[kp][:], z3g[g][:],
                                     start=True, stop=True)
                    nc.scalar.activation(h1[ci][:], p1[:], AF.Relu,
                                         bias=hb1w[:, kp:kp + 1])
                for ci, (g, kp) in enumerate(CI):
                    p2 = ps.tile([128, 512], F32, tag=f"b{ci}")
                    nc.tensor.matmul(p2[:], hw2p[kp][:], h1[ci][:],
                                     start=True, stop=True)
                    nc.vector.tensor_tensor(
                        out=hkt[ci][:], in0=p2[:],
                        in1=hb2w[:, kp:kp + 1].to_broadcast([128, 512])[:],
                        op=ALU.add)
                    nc.vector.tensor_scalar(out=hm2[ci][:], in0=hkt[ci][:],
                                            scalar1=-2.0, scalar2=None,
                                            op0=ALU.mult)
                    nc.vector.tensor_tensor(out=sqt[ci][:], in0=hkt[ci][:],
                                            in1=hkt[ci][:], op=ALU.mult)
                rsb = [[None, None] for _ in range(8)]
                for ci, (g, kp) in enumerate(CI):
                    for h in range(2):
                        prh = ps.tile([1, 512], F32, tag=f"b{(2 * ci + h) % 8}")
                        nc.tensor.matmul(prh[:], selm[:, h:h + 1], sqt[ci][:],
                                         start=True, stop=True)
                        rsb[ci][h] = fh.tile([1, 512], F32, tag=f"rs{ci}_{h}",
                                             name=f"rs{ci}_{h}")
                        nc.vector.tensor_copy(rsb[ci][h][:], prh[:])
                for ci, (g, kp) in enumerate(CI):
                    d2t = [None] * 8
                    for h in range(2):
                        for mb in range(4):
                            i8 = 4 * h + mb
                            pd = ps.tile([128, 512], F32, tag=f"b{i8}")
                            nc.tensor.matmul(
                                pd[:],
                                hm2[ci][64 * h:64 * h + 64,
                                        128 * mb:128 * mb + 128],
                                hkt[ci][64 * h:64 * h + 64, :],
                                start=True, stop=False)
                            nc.tensor.matmul(pd[:], onf[:, 0:128],
                                             rsb[ci][h][:], start=False,
                                             stop=False, skip_group_check=True)
                            nc.tensor.matmul(
                                pd[:], rsb[ci][h][:, 128 * mb:128 * mb + 128],
                                onf[:], start=False, stop=True,
                                skip_group_check=True)
                            d2t[i8] = fh.tile([128, 512], F32, tag=f"d2_{i8}",
                                              name=f"d2_{i8}")
                            nc.vector.tensor_scalar(out=d2t[i8][:], in0=pd[:],
                                                    scalar1=1e-12,
                                                    scalar2=None, op0=ALU.max)
                    lnt = [None] * 8
                    for i8 in range(8):
                        lnt[i8] = fh.tile([128, 512], F32, tag=f"ln_{i8}",
                                          name=f"ln_{i8}")
                        nc.scalar.activation(lnt[i8][:], d2t[i8][:], AF.Ln)
                    for i8 in range(8):
                        h, mb = i8 // 4, i8 % 4
                        qt = fh.tile([128, 512], F32, tag="qt", bufs=4)
                        nc.scalar.activation(qt[:], lnt[i8][:], AF.Sigmoid,
                                             bias=nla[:, 0:1], scale=-UMAP_B)
                        row = ((g * K + 2 * kp + h) * 4 + mb) * 128
                        nc.sync.dma_start(qout[row:row + 128, :], qt[:])
    nc.compile()
    return nc


def _host_prep(inputs):
    x = np.asarray(inputs["x"], np.float32)
    edge_index = np.asarray(inputs["edge_index"], np.int64)
    src, dst = edge_index[0], edge_index[1]
    hw1 = np.asarray(inputs["head_w1"], np.float32)
    hw2 = np.asarray(inputs["head_w2"], np.float32)
    hb1 = np.asarray(inputs["head_b1"], np.float32)
    hb2 = np.asarray(inputs["head_b2"], np.float32)

    hw1b = np.zeros((2 * ZI, 2 * ZI), np.float32)
    hw2b = np.zeros((4 * ZI, 2 * ZI), np.float32)
    hb1b = np.zeros((128, 2), np.float32)
    hb2b = np.zeros((128, 2), np.float32)
    for kp in range(2):
        hw1b[ZI * kp:ZI * kp + ZI, 0:ZI] = hw1[2 * kp]
        hw1b[ZI * kp:ZI * kp + ZI, ZI:2 * ZI] = hw1[2 * kp + 1]
        hw2b[128 * kp:128 * kp + ZI, 0:ZI] = hw2[2 * kp]
        hw2b[128 * kp + ZI:128 * kp + 128, ZI:2 * ZI] = hw2[2 * kp + 1]
        hb1b[0:ZI, kp] = hb1[2 * kp]
        hb1b[ZI:128, kp] = hb1[2 * kp + 1]
        hb2b[0:ZI, kp] = hb2[2 * kp]
        hb2b[ZI:128, kp] = hb2[2 * kp + 1]
    selm = np.zeros((128, 2), np.float32)
    selm[0:ZI, 0] = 1.0
    selm[ZI:128, 1] = 1.0

    shared = {
        "embw": np.ascontiguousarray(np.vstack(
            [np.asarray(inputs["emb_w"], np.float32),
             np.asarray(inputs["emb_b"], np.float32)[None, :]])),
        "gw1": np.ascontiguousarray(
            np.asarray(inputs["gin_w1"], np.float32).reshape(L * D, D)),
        "gw2": np.ascontiguousarray(
            np.asarray(inputs["gin_w2"], np.float32).reshape(L * D, D)),
        "mw1": np.ascontiguousarray(np.asarray(inputs["mlp_w1"], np.float32)),
        "mw2": np.ascontiguousarray(np.asarray(inputs["mlp_w2"], np.float32)),
        "mw3": np.ascontiguousarray(np.asarray(inputs["mlp_w3"], np.float32)),
        "hw1b": hw1b, "hw2b": hw2b,
        "hb1b_d": hb1b, "hb2b_d": hb2b, "selm_d": selm,
        "gb1_d": np.ascontiguousarray(
            np.asarray(inputs["gin_b1"], np.float32)
            .reshape(L, 4, 128).transpose(2, 0, 1).reshape(128, 16)),
        "bng_d": np.ascontiguousarray(
            np.asarray(inputs["bn_g"], np.float32)
            .reshape(L, 4, 128).transpose(2, 0, 1).reshape(128, 16)),
        "bnb_d": np.ascontiguousarray(
            np.asarray(inputs["bn_b"], np.float32)
            .reshape(L, 4, 128).transpose(2, 0, 1).reshape(128, 16)),
        "mb1_d": np.ascontiguousarray(
            np.asarray(inputs["mlp_b1"], np.float32).reshape(8, 128).T),
        "mb2_d": np.ascontiguousarray(
            np.asarray(inputs["mlp_b2"], np.float32).reshape(8, 128).T),
        "mb3_d": np.ascontiguousarray(
            np.asarray(inputs["mlp_b3"], np.float32)[:, None]),
    }

    # layer-0 exact aggregation operands
    emb_w = np.asarray(inputs["emb_w"], np.float32)
    emb_b = np.asarray(inputs["emb_b"], np.float32)
    xhi = x.astype(ml_dtypes.float8_e4m3)
    xlo = (x - xhi.astype(np.float32)).astype(ml_dtypes.float8_e4m3)
    xa = np.zeros((N, 32), ml_dtypes.float8_e4m3)
    xa[:, 0:9] = xhi
    xa[:, 10:19] = xlo
    xa[:, 20] = np.float32(1.0)
    shared["xg8"] = np.ascontiguousarray(
        xa.reshape(KB2, 2, 128, 32).transpose(2, 0, 1, 3))
    ewa = np.zeros((32, D), np.float32)
    ewa[0:9] = emb_w
    ewa[10:19] = emb_w
    ewa[20] = emb_b
    shared["embwa"] = ewa

    in_maps = []
    ones_row = np.ones((1, NL), np.float32)
    for c in range(NCORES):
        lo = NL * c
        mask = (dst >= lo) & (dst < lo + NL)
        flat = src[mask] * NL + (dst[mask] - lo)
        a = np.bincount(flat, minlength=N * NL).astype(np.float32)
        # paired layout for DoubleRow: [p, kb2, two, dst]
        a = a.reshape(KB2, 2, 128, NL).transpose(2, 0, 1, 3)
        m = dict(shared)
        m["acm8"] = np.ascontiguousarray(a).astype(ml_dtypes.float8_e4m3)
        m["xt"] = np.ascontiguousarray(
            np.vstack([x[lo:lo + NL].T, ones_row]))
        in_maps.append(m)
    return in_maps


def kernel(**inputs) -> np.ndarray:
    global _NC_CACHE
    if _NC_CACHE is None:
        _NC_CACHE = build_nc()
    nc = _NC_CACHE
    in_maps = _host_prep(inputs)
    res = run_bass_kernel_spmd(nc, in_maps, core_ids=list(range(NCORES)))
    out = np.concatenate(
        [np.asarray(res.results[c]["qout"]).reshape(GL, K, NG, NG)
         for c in range(NCORES)], axis=0)
    return out


# revision 24
# speedup vs baseline: 1.6478x; 1.0628x over previous
import sys
sys.path.insert(0, "/opt/trn_rl_repo")
import math
import numpy as np
import ml_dtypes

import concourse.bass as bass
from concourse import bacc, mybir
from concourse.tile import TileContext
from concourse.bass_utils import run_bass_kernel_spmd
from concourse.masks import make_identity

F32 = mybir.dt.float32
F32R = mybir.dt.float32r
F8 = mybir.dt.float8e4
AF = mybir.ActivationFunctionType
ALU = mybir.AluOpType
AX = mybir.AxisListType
DR = mybir.MatmulPerfMode.DoubleRow

N, G, E = 16384, 32, 524288
D, DFF, ZI, K, L = 512, 1024, 64, 4, 4
UMAP_A, UMAP_B = 1.577, 0.8951
BN_EPS = 1e-5
NCORES = 8
NL = N // NCORES      # 2048 local nodes per core
GL = G // NCORES      # 4 local graphs per core
NG = N // G           # 512 nodes per graph
KB2 = N // 256        # 64 paired source blocks (256 src nodes each)
RG = [list(range(NCORES))]

_NC_CACHE = None


def build_nc():
    nc = bacc.Bacc("TRN2", target_bir_lowering=False, debug=False,
                   enable_asserts=True, num_devices=NCORES)

    xt = nc.dram_tensor("xt", (10, NL), F32R, kind="ExternalInput")
    # full-graph x in paired fp8: cols = [x_hi(10) | x_lo(10) | ones(1) | pad]
    xg8 = nc.dram_tensor("xg8", (128, KB2, 2, 32), F8, kind="ExternalInput")
    # emb_w stacked for the hi/lo recombine: rows = [W, W, b, 0...]
    embwa = nc.dram_tensor("embwa", (32, D), F32R, kind="ExternalInput")
    acm8 = nc.dram_tensor("acm8", (128, KB2, 2, NL), F8, kind="ExternalInput")
    embw = nc.dram_tensor("embw", (10, D), F32R, kind="ExternalInput")
    gw1 = nc.dram_tensor("gw1", (L * D, D), F32R, kind="ExternalInput")
    gw2 = nc.dram_tensor("gw2", (L * D, D), F32R, kind="ExternalInput")
    mw1 = nc.dram_tensor("mw1", (D, DFF), F32R, kind="ExternalInput")
    mw2 = nc.dram_tensor("mw2", (DFF, DFF), F32R, kind="ExternalInput")
    mw3 = nc.dram_tensor("mw3", (DFF, ZI), F32R, kind="ExternalInput")
    # head weights batched in pairs: hw1b[64*kp:64*kp+64] = [w1_{2kp}|w1_{2kp+1}]
    hw1b = nc.dram_tensor("hw1b", (2 * ZI, 2 * ZI), F32R, kind="ExternalInput")
    # hw2b[128*kp:...] = blockdiag(w2_{2kp}, w2_{2kp+1})
    hw2b = nc.dram_tensor("hw2b", (2 * 2 * ZI, 2 * ZI), F32R,
                          kind="ExternalInput")
    gb1_d = nc.dram_tensor("gb1_d", (128, 16), F32, kind="ExternalInput")
    bng_d = nc.dram_tensor("bng_d", (128, 16), F32, kind="ExternalInput")
    bnb_d = nc.dram_tensor("bnb_d", (128, 16), F32, kind="ExternalInput")
    mb1_d = nc.dram_tensor("mb1_d", (128, 8), F32, kind="ExternalInput")
    mb2_d = nc.dram_tensor("mb2_d", (128, 8), F32, kind="ExternalInput")
    mb3_d = nc.dram_tensor("mb3_d", (ZI, 1), F32, kind="ExternalInput")
    hb1b_d = nc.dram_tensor("hb1b_d", (128, 2), F32, kind="ExternalInput")
    hb2b_d = nc.dram_tensor("hb2b_d", (128, 2), F32, kind="ExternalInput")
    selm_d = nc.dram_tensor("selm_d", (128, 2), F32, kind="ExternalInput")
    qout = nc.dram_tensor("qout", (GL * K * 4 * 128, NG), F32,
                          kind="ExternalOutput")

    with TileContext(nc) as tc:
        with (
            tc.tile_pool(name="const", bufs=1) as cp,
            tc.tile_pool(name="res", bufs=1) as rp,
            tc.tile_pool(name="ps", bufs=1, space="PSUM") as ps,
            tc.tile_pool(name="dram", bufs=1, space="DRAM") as dp,
        ):
            ident = cp.tile([128, 128], F32, tag="ident")
            make_identity(nc, ident[:])
            nla = cp.tile([128, 1], F32, tag="nla")
            nc.gpsimd.memset(nla[:], -math.log(UMAP_A))
            onf = cp.tile([1, NG], F32, tag="onf")
            nc.gpsimd.memset(onf[:], 1.0)

            gb1w = cp.tile([128, 16], F32, tag="gb1w")
            nc.sync.dma_start(gb1w[:], gb1_d[:, :])
            bngw = cp.tile([128, 16], F32, tag="bngw")
            nc.sync.dma_start(bngw[:], bng_d[:, :])
            bnbw = cp.tile([128, 16], F32, tag="bnbw")
            nc.sync.dma_start(bnbw[:], bnb_d[:, :])
            mb1w = cp.tile([128, 8], F32, tag="mb1w")
            nc.sync.dma_start(mb1w[:], mb1_d[:, :])
            mb2w = cp.tile([128, 8], F32, tag="mb2w")
            nc.sync.dma_start(mb2w[:], mb2_d[:, :])
            mb3w = cp.tile([ZI, 1], F32, tag="mb3w")
            nc.sync.dma_start(mb3w[:], mb3_d[:, :])
            hb1w = cp.tile([128, 2], F32, tag="hb1w")
            nc.sync.dma_start(hb1w[:], hb1b_d[:, :])
            hb2w = cp.tile([128, 2], F32, tag="hb2w")
            nc.sync.dma_start(hb2w[:], hb2b_d[:, :])
            selm = cp.tile([128, 2], F32, tag="selm")
            nc.sync.dma_start(selm[:], selm_d[:, :])

            hT = [rp.tile([128, NL], F32R, tag=f"hT{fc}", name=f"hT{fc}")
                  for fc in range(4)]

            # chunked hi/lo tables: A covers local kb2 0-3, B covers 4-7
            sA = [dp.tile([4, 128, 2, 2 * D], F8, tag=f"sA{i}", name=f"sA{i}")
                  for i in range(L)]
            sB = [dp.tile([4, 128, 2, 2 * D], F8, tag=f"sB{i}", name=f"sB{i}")
                  for i in range(L)]
            tA = [dp.tile([KB2 // 2, 128, 2, 2 * D], F8, tag=f"tA{i}",
                          name=f"tA{i}", addr_space="Shared")
                  for i in range(L)]
            tB = [dp.tile([KB2 // 2, 128, 2, 2 * D], F8, tag=f"tB{i}",
                          name=f"tB{i}", addr_space="Shared")
                  for i in range(L)]
            bn_loc = [dp.tile([128, 8], F32, tag=f"bl{i}", name=f"bl{i}")
                      for i in range(L)]
            bn_glob = [dp.tile([128, 8], F32, tag=f"bg{i}", name=f"bg{i}",
                               addr_space="Shared")
                       for i in range(L)]
            war_l = dp.tile([128, 8], F32, name="war_l")
            war_g = dp.tile([128, 8], F32, name="war_g", addr_space="Shared")

            # ---------------- embedding + collective warmup ----------------
            with tc.tile_pool(name="emb", bufs=1) as ep:
                xt_sb = ep.tile([10, NL], F32R, tag="xt")
                nc.sync.dma_start(xt_sb[:], xt[:, :])
                ew_sb = ep.tile([10, D], F32R, tag="ew")
                nc.sync.dma_start(ew_sb[:], embw[:, :])
                wz = ep.tile([128, 8], F32, tag="wz")
                nc.gpsimd.memset(wz[:], 0.0)
                nc.sync.dma_start(war_l[:, :], wz[:])
                nc.gpsimd.collective_compute(
                    "AllReduce", ALU.add, ins=[war_l[:, :].opt()],
                    outs=[war_g[:, :].opt()], replica_groups=RG)
                for fc in range(4):
                    for j in range(4):
                        p = ps.tile([128, 512], F32, tag=f"b{4 + fc}")
                        nc.tensor.matmul(p[:], ew_sb[:, 128 * fc:128 * fc + 128],
                                         xt_sb[:, 512 * j:512 * j + 512],
                                         start=True, stop=True)
                        nc.vector.tensor_copy(hT[fc][:, 512 * j:512 * j + 512],
                                              p[:])

            # ---------------- GIN layers ----------------
            with tc.tile_pool(name="gin", bufs=1) as gp:
                for l in range(L):
                    w1s = gp.tile([128, 2048], F32R, tag="w1")
                    w2s = gp.tile([128, 2048], F32R, tag="w2")
                    for ic in range(4):
                        r0 = 512 * l + 128 * ic
                        nc.sync.dma_start(w1s[:, 512 * ic:512 * ic + 512],
                                          gw1[r0:r0 + 128, :])
                        nc.sync.dma_start(w2s[:, 512 * ic:512 * ic + 512],
                                          gw2[r0:r0 + 128, :])

                    mt = [gp.tile([128, NL], F32R, tag=f"mt{fc}",
                                  name=f"mt{fc}_{l}") for fc in range(4)]

                    if l == 0:
                        # layer-0 aggregation, exact: agg0 = (A.T xa) @ Wa
                        xg_sb = gp.tile([128, KB2, 2, 32], F8, tag="xg")
                        nc.sync.dma_start(xg_sb[:], xg8[:, :, :, :])
                        ewa = gp.tile([32, D], F32R, tag="ewa")
                        nc.sync.dma_start(ewa[:], embwa[:, :])
                        px = [ps.tile([32, 512], F32, tag=f"b{i}",
                                      name=f"px{i}") for i in range(4)]
                        for half in range(2):
                            for kb2 in range(KB2):
                                a8 = gp.tile([128, 2, 1024], F8, tag="a8",
                                             bufs=4)
                                eng = nc.sync if kb2 % 2 == 0 else nc.scalar
                                eng.dma_start(
                                    a8[:],
                                    acm8[:, kb2, :,
                                         1024 * half:1024 * half + 1024])
                                lhs = xg_sb[:, kb2, :, :]
                                for dc in range(2):
                                    nc.tensor.matmul(
                                        px[2 * half + dc][:], lhs,
                                        a8[:, :, 512 * dc:512 * dc + 512],
                                        start=(kb2 == 0), stop=(kb2 == KB2 - 1),
                                        perf_mode=DR)
                        atxt = gp.tile([32, NL], F32R, tag="atxt")
                        for dt in range(4):
                            nc.vector.tensor_copy(
                                atxt[:, 512 * dt:512 * dt + 512], px[dt][:])
                        for d in range(4):
                            for jc in range(4):
                                pr0 = ps.tile([128, 512], F32, tag=f"b{4 + jc}")
                                nc.tensor.matmul(
                                    pr0[:], ewa[:, 128 * d:128 * d + 128],
                                    atxt[:, 512 * jc:512 * jc + 512],
                                    start=True, stop=True)
                                nc.vector.tensor_tensor(
                                    out=mt[d][:, 512 * jc:512 * jc + 512],
                                    in0=pr0[:],
                                    in1=hT[d][:, 512 * jc:512 * jc + 512],
                                    op=ALU.add)
                    else:
                        # aggregation: aggT = (hi + lo).T @ A8 (DoubleRow) + hT
                        # consume chunk-A table rows first, then chunk-B
                        order = ([("A", r) for r in range(KB2 // 2)]
                                 + [("B", r) for r in range(KB2 // 2)])
                        for half in range(2):
                            pb = [ps.tile([128, 512], F32, tag=f"b{i}",
                                          name=f"pb{i}") for i in range(8)]
                            for ii, (ab, r) in enumerate(order):
                                gkb2 = 8 * (r // 4) + (r % 4) + (4 if ab == "B"
                                                                 else 0)
                                a8 = gp.tile([128, 2, 1024], F8, tag="a8",
                                             bufs=4)
                                nc.sync.dma_start(
                                    a8[:],
                                    acm8[:, gkb2, :,
                                         1024 * half:1024 * half + 1024])
                                hk8 = gp.tile([128, 2, 1024], F8, tag="hk8",
                                              bufs=4)
                                tab = tA[l] if ab == "A" else tB[l]
                                nc.scalar.dma_start(hk8[:], tab[r, :, :, :])
                                for d in range(4):
                                    for hilo in range(2):
                                        lhs = hk8[:, :,
                                                  512 * hilo + 128 * d:
                                                  512 * hilo + 128 * d + 128]
                                        for dc in range(2):
                                            nc.tensor.matmul(
                                                pb[2 * d + dc][:], lhs,
                                                a8[:, :,
                                                   512 * dc:512 * dc + 512],
                                                start=(ii == 0 and hilo == 0),
                                                stop=(ii == KB2 - 1
                                                      and hilo == 1),
                                                perf_mode=DR)
                            for d in range(4):
                                for dc in range(2):
                                    col = 1024 * half + 512 * dc
                                    nc.vector.tensor_tensor(
                                        out=mt[d][:, col:col + 512],
                                        in0=pb[2 * d + dc][:],
                                        in1=hT[d][:, col:col + 512],
                                        op=ALU.add)

                    # GIN MLP: u1 = relu(m@w1+b1); u2 = u1@w2 (into mt)
                    # fold BN partial stats in as u2 chunks are produced
                    st16s = gp.tile([128, 16], F32, tag="st16s")
                    st16q = gp.tile([128, 16], F32, tag="st16q")
                    for j in range(4):
                        ncol = 512 * j
                        u1c = [gp.tile([128, 512], F32R, tag=f"u1_{oc}", bufs=2,
                                       name=f"u1c{oc}") for oc in range(4)]
                        for oc in range(4):
                            p = ps.tile([128, 512], F32, tag=f"b{oc}")
                            for ic in range(4):
                                nc.tensor.matmul(
                                    p[:],
                                    w1s[:, 512 * ic + 128 * oc:
                                        512 * ic + 128 * oc + 128],
                                    mt[ic][:, ncol:ncol + 512],
                                    start=(ic == 0), stop=(ic == 3))
                            nc.scalar.activation(
                                u1c[oc][:], p[:], AF.Relu,
                                bias=gb1w[:, 4 * l + oc:4 * l + oc + 1])
                        for oc in range(4):
                            p = ps.tile([128, 512], F32, tag=f"b{4 + oc}")
                            for ic in range(4):
                                nc.tensor.matmul(
                                    p[:],
                                    w2s[:, 512 * ic + 128 * oc:
                                        512 * ic + 128 * oc + 128],
                                    u1c[ic][:],
                                    start=(ic == 0), stop=(ic == 3))
                            nc.vector.tensor_copy(mt[oc][:, ncol:ncol + 512],
                                                  p[:])
                            col = 4 * j + oc
                            sq = gp.tile([128, 512], F32, tag="sq", bufs=2)
                            nc.scalar.activation(sq[:], p[:], AF.Square)
                            nc.vector.reduce_sum(st16s[:, col:col + 1],
                                                 mt[oc][:, ncol:ncol + 512],
                                                 axis=AX.X)
                            nc.vector.reduce_sum(st16q[:, col:col + 1], sq[:],
                                                 axis=AX.X)

                    # assemble + AllReduce BN stats
                    stat = gp.tile([128, 8], F32, tag="stat")
                    t4a = gp.tile([128, 4], F32, tag="t4a")
                    t4b = gp.tile([128, 4], F32, tag="t4b")
                    nc.vector.tensor_tensor(out=t4a[:], in0=st16s[:, 0:4],
                                            in1=st16s[:, 4:8], op=ALU.add)
                    nc.vector.tensor_tensor(out=t4b[:], in0=st16s[:, 8:12],
                                            in1=st16s[:, 12:16], op=ALU.add)
                    nc.vector.tensor_tensor(out=stat[:, 0:4], in0=t4a[:],
                                            in1=t4b[:], op=ALU.add)
                    t4c = gp.tile([128, 4], F32, tag="t4c")
                    t4d = gp.tile([128, 4], F32, tag="t4d")
                    nc.vector.tensor_tensor(out=t4c[:], in0=st16q[:, 0:4],
                                            in1=st16q[:, 4:8], op=ALU.add)
                    nc.vector.tensor_tensor(out=t4d[:], in0=st16q[:, 8:12],
                                            in1=st16q[:, 12:16], op=ALU.add)
                    nc.vector.tensor_tensor(out=stat[:, 4:8], in0=t4c[:],
                                            in1=t4d[:], op=ALU.add)
                    nc.sync.dma_start(bn_loc[l][:, :], stat[:])
                    nc.gpsimd.collective_compute(
                        "AllReduce", ALU.add, ins=[bn_loc[l][:, :].opt()],
                        outs=[bn_glob[l][:, :].opt()], replica_groups=RG)
                    ga = gp.tile([128, 8], F32, tag="ga")
                    nc.sync.dma_start(ga[:], bn_glob[l][:, :])

                    # BN coefficients, vectorized over the 4 feature blocks
                    mu4 = gp.tile([128, 4], F32, tag="mu4")
                    nc.vector.tensor_scalar(out=mu4[:], in0=ga[:, 0:4],
                                            scalar1=1.0 / N, scalar2=None,
                                            op0=ALU.mult)
                    ex24 = gp.tile([128, 4], F32, tag="ex24")
                    nc.vector.tensor_scalar(out=ex24[:], in0=ga[:, 4:8],
                                            scalar1=1.0 / N, scalar2=None,
                                            op0=ALU.mult)
                    mu2 = gp.tile([128, 4], F32, tag="mu2")
                    nc.vector.tensor_tensor(out=mu2[:], in0=mu4[:], in1=mu4[:],
                                            op=ALU.mult)
                    var4 = gp.tile([128, 4], F32, tag="var4")
                    nc.vector.tensor_tensor(out=var4[:], in0=ex24[:],
                                            in1=mu2[:], op=ALU.subtract)
                    vare = gp.tile([128, 4], F32, tag="vare")
                    nc.vector.tensor_scalar(out=vare[:], in0=var4[:],
                                            scalar1=BN_EPS, scalar2=None,
                                            op0=ALU.add)
                    std4 = gp.tile([128, 4], F32, tag="std4")
                    nc.scalar.activation(std4[:], vare[:], AF.Sqrt)
                    inv4 = gp.tile([128, 4], F32, tag="inv4")
                    nc.vector.reciprocal(inv4[:], std4[:])
                    sv4 = gp.tile([128, 4], F32, tag="sv4")
                    nc.vector.tensor_tensor(out=sv4[:], in0=inv4[:],
                                            in1=bngw[:, 4 * l:4 * l + 4],
                                            op=ALU.mult)
                    mst = gp.tile([128, 4], F32, tag="mst")
                    nc.vector.tensor_tensor(out=mst[:], in0=mu4[:], in1=sv4[:],
                                            op=ALU.mult)
                    tv4 = gp.tile([128, 4], F32, tag="tv4")
                    nc.vector.tensor_tensor(out=tv4[:],
                                            in0=bnbw[:, 4 * l:4 * l + 4],
                                            in1=mst[:], op=ALU.subtract)

                    # BN apply + residual, j-outer so transposes start early
                    for j in range(4):
                        ncol = 512 * j
                        for fc in range(4):
                            rt = gp.tile([128, 512], F32R, tag=f"rt{fc}",
                                         bufs=2, name=f"rt{fc}")
                            nc.scalar.activation(
                                rt[:], mt[fc][:, ncol:ncol + 512], AF.Relu,
                                bias=tv4[:, fc:fc + 1], scale=sv4[:, fc:fc + 1])
                            nc.vector.tensor_tensor(
                                out=hT[fc][:, ncol:ncol + 512], in0=rt[:],
                                in1=hT[fc][:, ncol:ncol + 512], op=ALU.add)
                        if l < L - 1:
                            for nb in range(4 * j, 4 * j + 4):
                                hn2 = gp.tile([128, 2 * 512], F8, tag="hn2",
                                              bufs=2)
                                for fc in range(4):
                                    pt = ps.tile([128, 128], F32, tag=f"b{fc}")
                                    nc.tensor.transpose(
                                        pt[:],
                                        hT[fc][:, 128 * nb:128 * nb + 128]
                                        .bitcast(F32),
                                        ident[:])
                                    nc.vector.tensor_copy(
                                        hn2[:, 128 * fc:128 * fc + 128], pt[:])
                                    h32 = gp.tile([128, 128], F32, tag="h32",
                                                  bufs=2)
                                    nc.vector.tensor_copy(
                                        h32[:],
                                        hn2[:, 128 * fc:128 * fc + 128])
                                    nc.vector.tensor_tensor(
                                        out=hn2[:, 512 + 128 * fc:
                                                512 + 128 * fc + 128],
                                        in0=pt[:], in1=h32[:],
                                        op=ALU.subtract)
                                sl = sA[l + 1] if nb < 8 else sB[l + 1]
                                nc.sync.dma_start(
                                    sl[(nb % 8) // 2, :, nb % 2, :], hn2[:])
                            if j == 1:
                                nc.gpsimd.collective_compute(
                                    "AllGather", ALU.bypass,
                                    ins=[sA[l + 1][:, :, :, :].opt()],
                                    outs=[tA[l + 1][:, :, :, :].opt()],
                                    replica_groups=RG)
                            if j == 3:
                                nc.gpsimd.collective_compute(
                                    "AllGather", ALU.bypass,
                                    ins=[sB[l + 1][:, :, :, :].opt()],
                                    outs=[tB[l + 1][:, :, :, :].opt()],
                                    replica_groups=RG)

            # ---------------- final MLP + heads + pairwise ----------------
            z3g = [rp.tile([ZI, 512], F32R, tag=f"z3_{g}", name=f"z3_{g}")
                   for g in range(GL)]
            with tc.tile_pool(name="finz", bufs=1) as fz:
                mwa = [fz.tile([128, DFF], F32R, tag=f"mw1_{ic}", name=f"mwa{ic}")
                       for ic in range(4)]
                for ic in range(4):
                    nc.sync.dma_start(mwa[ic][:],
                                      mw1[128 * ic:128 * ic + 128, :])
                mwb = [fz.tile([128, DFF], F32R, tag=f"mw2_{ic}", name=f"mwb{ic}")
                       for ic in range(8)]
                for ic in range(8):
                    nc.sync.dma_start(mwb[ic][:],
                                      mw2[128 * ic:128 * ic + 128, :])
                mwc = [fz.tile([128, ZI], F32R, tag=f"mw3_{ic}", name=f"mwc{ic}")
                       for ic in range(8)]
                for ic in range(8):
                    nc.sync.dma_start(mwc[ic][:],
                                      mw3[128 * ic:128 * ic + 128, :])
                for g in range(GL):
                    gcol = 512 * g
                    z1 = [fz.tile([128, 512], F32R, tag=f"z1_{oc}", bufs=2,
                                  name=f"z1_{oc}") for oc in range(8)]
                    for oc in range(8):
                        p = ps.tile([128, 512], F32, tag=f"b{oc}")
                        for ic in range(4):
                            nc.tensor.matmul(
                                p[:],
                                mwa[ic][:, 128 * oc:128 * oc + 128],
                                hT[ic][:, gcol:gcol + 512],
                                start=(ic == 0), stop=(ic == 3))
                        nc.scalar.activation(z1[oc][:], p[:], AF.Relu,
                                             bias=mb1w[:, oc:oc + 1])
                    z2 = [fz.tile([128, 512], F32R, tag=f"z2_{oc}", bufs=2,
                                  name=f"z2_{oc}") for oc in range(8)]
                    for oc in range(8):
                        p = ps.tile([128, 512], F32, tag=f"b{oc}")
                        for ic in range(8):
                            nc.tensor.matmul(
                                p[:],
                                mwb[ic][:, 128 * oc:128 * oc + 128],
                                z1[ic][:],
                                start=(ic == 0), stop=(ic == 7))
                        nc.scalar.activation(z2[oc][:], p[:], AF.Relu,
                                             bias=mb2w[:, oc:oc + 1])
                    pz = ps.tile([ZI, 512], F32, tag="b0")
                    for ic in range(8):
                        nc.tensor.matmul(pz[:], mwc[ic][:, 0:ZI], z2[ic][:],
                                         start=(ic == 0), stop=(ic == 7))
                    nc.vector.tensor_tensor(
                        out=z3g[g][:], in0=pz[:],
                        in1=mb3w[:, 0:1].to_broadcast([ZI, 512])[:],
                        op=ALU.add)

            # heads stage: 8 chains (g, kp); pairwise in split-bf16
            with tc.tile_pool(name="finh", bufs=1) as fh:
                BF = mybir.dt.bfloat16
                hw1s = fh.tile([ZI, 2 * ZI], F32R, tag="hw1s")
                nc.sync.dma_start(hw1s[:], hw1b[0:ZI, :])
                hw1s2 = fh.tile([ZI, 2 * ZI], F32R, tag="hw1s2")
                nc.sync.dma_start(hw1s2[:], hw1b[ZI:2 * ZI, :])
                hw2s = fh.tile([128, 2 * ZI], F32R, tag="hw2s")
                nc.sync.dma_start(hw2s[:], hw2b[0:128, :])
                hw2s2 = fh.tile([128, 2 * ZI], F32R, tag="hw2s2")
                nc.sync.dma_start(hw2s2[:], hw2b[128:256, :])
                hw1p = [hw1s, hw1s2]
                hw2p = [hw2s, hw2s2]
                selmb = fh.tile([128, 2], BF, tag="selmb")
                nc.vector.tensor_copy(selmb[:], selm[:])
                onfb = fh.tile([1, NG], BF, tag="onfb")
                nc.vector.tensor_copy(onfb[:], onf[:])
                CI = [(g, kp) for kp in range(2) for g in range(GL)]
                hA = [fh.tile([128, 512], BF, tag=f"hA{ci}", name=f"hA{ci}")
                      for ci in range(8)]
                hB = [fh.tile([128, 512], BF, tag=f"hB{ci}", name=f"hB{ci}")
                      for ci in range(8)]
                mA = [fh.tile([128, 512], BF, tag=f"mA{ci}", name=f"mA{ci}")
                      for ci in range(8)]
                mB = [fh.tile([128, 512], BF, tag=f"mB{ci}", name=f"mB{ci}")
                      for ci in range(8)]
                sA8 = [fh.tile([128, 512], BF, tag=f"sA8{ci}", name=f"sA8{ci}")
                       for ci in range(8)]
                sB8 = [fh.tile([128, 512], BF, tag=f"sB8{ci}", name=f"sB8{ci}")
                       for ci in range(8)]
                # prep: head MLP then bf16 splits
                for ci, (g, kp) in enumerate(CI):
                    p1 = ps.tile([128, 512], F32, tag=f"b{ci}")
                    nc.tensor.matmul(p1[:], hw1p[kp][:], z3g[g][:],
                                     start=True, stop=True)
                    h1 = fh.tile([128, 512], F32R, tag="h1", bufs=2)
                    nc.scalar.activation(h1[:], p1[:], AF.Relu,
                                         bias=hb1w[:, kp:kp + 1])
                    p2 = ps.tile([128, 512], F32, tag=f"b{ci}")
                    nc.tensor.matmul(p2[:], hw2p[kp][:], h1[:],
                                     start=True, stop=True)
                    hkt = fh.tile([128, 512], F32, tag="hkt", bufs=2)
                    nc.vector.tensor_tensor(
                        out=hkt[:], in0=p2[:],
                        in1=hb2w[:, kp:kp + 1].to_broadcast([128, 512])[:],
                        op=ALU.add)
                    nc.vector.tensor_copy(hA[ci][:], hkt[:])
                    nc.vector.tensor_tensor(out=hB[ci][:], in0=hkt[:],
                                            in1=hA[ci][:], op=ALU.subtract)
                    nc.vector.tensor_scalar(out=mA[ci][:], in0=hA[ci][:],
                                            scalar1=-2.0, scalar2=None,
                                            op0=ALU.mult)
                    nc.vector.tensor_scalar(out=mB[ci][:], in0=hB[ci][:],
                                            scalar1=-2.0, scalar2=None,
                                            op0=ALU.mult)
                    hts = fh.tile([128, 512], F32, tag="hts", bufs=2)
                    nc.vector.tensor_tensor(out=hts[:], in0=hA[ci][:],
                                            in1=hB[ci][:], op=ALU.add)
                    sqt = fh.tile([128, 512], F32, tag="sqt", bufs=2)
                    nc.vector.tensor_tensor(out=sqt[:], in0=hts[:],
                                            in1=hts[:], op=ALU.mult)
                    nc.vector.tensor_copy(sA8[ci][:], sqt[:])
                    nc.vector.tensor_tensor(out=sB8[ci][:], in0=sqt[:],
                                            in1=sA8[ci][:], op=ALU.subtract)
                # pairwise
                for ci, (g, kp) in enumerate(CI):
                    d2t = [None] * 8
                    for h in range(2):
                        prh = ps.tile([1, 512], F32, tag=f"b{4 * h}")
                        nc.tensor.matmul(prh[:], selmb[:, h:h + 1], sA8[ci][:],
                                         start=True, stop=False)
                        nc.tensor.matmul(prh[:], selmb[:, h:h + 1], sB8[ci][:],
                                         start=False, stop=True,
                                         skip_group_check=True)
                        rsb = fh.tile([1, 512], F32, tag=f"rsb{h}", bufs=2,
                                      name=f"rsb{h}")
                        nc.vector.tensor_copy(rsb[:], prh[:])
                        rA = fh.tile([1, 512], BF, tag=f"rA{h}", bufs=2,
                                     name=f"rA{h}")
                        nc.vector.tensor_copy(rA[:], rsb[:])
                        rB = fh.tile([1, 512], BF, tag=f"rB{h}", bufs=2,
                                     name=f"rB{h}")
                        nc.vector.tensor_tensor(out=rB[:], in0=rsb[:],
                                                in1=rA[:], op=ALU.subtract)
                        for mb in range(4):
                            i8 = 4 * h + mb
                            pd = ps.tile([128, 512], F32, tag=f"b{i8}")
                            nc.tensor.matmul(
                                pd[:],
                                mA[ci][64 * h:64 * h + 64,
                                       128 * mb:128 * mb + 128],
                                hA[ci][64 * h:64 * h + 64, :],
                                start=True, stop=False)
                            nc.tensor.matmul(
                                pd[:],
                                mA[ci][64 * h:64 * h + 64,
                                       128 * mb:128 * mb + 128],
                                hB[ci][64 * h:64 * h + 64, :],
                                start=False, stop=False, skip_group_check=True)
                            nc.tensor.matmul(
                                pd[:],
                                mB[ci][64 * h:64 * h + 64,
                                       128 * mb:128 * mb + 128],
                                hA[ci][64 * h:64 * h + 64, :],
                                start=False, stop=False, skip_group_check=True)
                            nc.tensor.matmul(pd[:], onfb[:, 0:128], rA[:],
                                             start=False, stop=False,
                                             skip_group_check=True)
                            nc.tensor.matmul(pd[:], onfb[:, 0:128], rB[:],
                                             start=False, stop=False,
                                             skip_group_check=True)
                            nc.tensor.matmul(pd[:],
                                             rA[:, 128 * mb:128 * mb + 128],
                                             onfb[:], start=False, stop=False,
                                             skip_group_check=True)
                            nc.tensor.matmul(pd[:],
                                             rB[:, 128 * mb:128 * mb + 128],
                                             onfb[:], start=False, stop=True,
                                             skip_group_check=True)
                            d2t[i8] = fh.tile([128, 512], F32, tag=f"d2_{i8}",
                                              name=f"d2_{i8}")
                            nc.vector.tensor_scalar(out=d2t[i8][:], in0=pd[:],
                                                    scalar1=1e-12,
                                                    scalar2=None, op0=ALU.max)
                    lnt = [None] * 8
                    for i8 in range(8):
                        lnt[i8] = fh.tile([128, 512], F32, tag=f"ln_{i8}",
                                          name=f"ln_{i8}")
                        nc.scalar.activation(lnt[i8][:], d2t[i8][:], AF.Ln)
                    for i8 in range(8):
                        h, mb = i8 // 4, i8 % 4
                        qt = fh.tile([128, 512], F32, tag="qt", bufs=4)
                        nc.scalar.activation(qt[:], lnt[i8][:], AF.Sigmoid,
                                             bias=nla[:, 0:1], scale=-UMAP_B)
                        row = ((g * K + 2 * kp + h) * 4 + mb) * 128
                        nc.sync.dma_start(qout[row:row + 128, :], qt[:])
    nc.compile()
    return nc


def _host_prep(inputs):
    x = np.asarray(inputs["x"], np.float32)
    edge_index = np.asarray(inputs["edge_index"], np.int64)
    src, dst = edge_index[0], edge_index[1]
    hw1 = np.asarray(inputs["head_w1"], np.float32)
    hw2 = np.asarray(inputs["head_w2"], np.float32)
    hb1 = np.asarray(inputs["head_b1"], np.float32)
    hb2 = np.asarray(inputs["head_b2"], np.float32)

    hw1b = np.zeros((2 * ZI, 2 * ZI), np.float32)
    hw2b = np.zeros((4 * ZI, 2 * ZI), np.float32)
    hb1b = np.zeros((128, 2), np.float32)
    hb2b = np.zeros((128, 2), np.float32)
    for kp in range(2):
        hw1b[ZI * kp:ZI * kp + ZI, 0:ZI] = hw1[2 * kp]
        hw1b[ZI * kp:ZI * kp + ZI, ZI:2 * ZI] = hw1[2 * kp + 1]
        hw2b[128 * kp:128 * kp + ZI, 0:ZI] = hw2[2 * kp]
        hw2b[128 * kp + ZI:128 * kp + 128, ZI:2 * ZI] = hw2[2 * kp + 1]
        hb1b[0:ZI, kp] = hb1[2 * kp]
        hb1b[ZI:128, kp] = hb1[2 * kp + 1]
        hb2b[0:ZI, kp] = hb2[2 * kp]
        hb2b[ZI:128, kp] = hb2[2 * kp + 1]
    selm = np.zeros((128, 2), np.float32)
    selm[0:ZI, 0] = 1.0
    selm[ZI:128, 1] = 1.0

    shared = {
        "embw": np.ascontiguousarray(np.vstack(
            [np.asarray(inputs["emb_w"], np.float32),
             np.asarray(inputs["emb_b"], np.float32)[None, :]])),
        "gw1": np.ascontiguousarray(
            np.asarray(inputs["gin_w1"], np.float32).reshape(L * D, D)),
        "gw2": np.ascontiguousarray(
            np.asarray(inputs["gin_w2"], np.float32).reshape(L * D, D)),
        "mw1": np.ascontiguousarray(np.asarray(inputs["mlp_w1"], np.float32)),
        "mw2": np.ascontiguousarray(np.asarray(inputs["mlp_w2"], np.float32)),
        "mw3": np.ascontiguousarray(np.asarray(inputs["mlp_w3"], np.float32)),
        "hw1b": hw1b, "hw2b": hw2b,
        "hb1b_d": hb1b, "hb2b_d": hb2b, "selm_d": selm,
        "gb1_d": np.ascontiguousarray(
            np.asarray(inputs["gin_b1"], np.float32)
            .reshape(L, 4, 128).transpose(2, 0, 1).reshape(128, 16)),
        "bng_d": np.ascontiguousarray(
            np.asarray(inputs["bn_g"], np.float32)
            .reshape(L, 4, 128).transpose(2, 0, 1).reshape(128, 16)),
        "bnb_d": np.ascontiguousarray(
            np.asarray(inputs["bn_b"], np.float32)
            .reshape(L, 4, 128).transpose(2, 0, 1).reshape(128, 16)),
        "mb1_d": np.ascontiguousarray(
            np.asarray(inputs["mlp_b1"], np.float32).reshape(8, 128).T),
        "mb2_d": np.ascontiguousarray(
            np.asarray(inputs["mlp_b2"], np.float32).reshape(8, 128).T),
        "mb3_d": np.ascontiguousarray(
            np.asarray(inputs["mlp_b3"], np.float32)[:, None]),
    }

    # layer-0 exact aggregation operands
    emb_w = np.asarray(inputs["emb_w"], np.float32)
    emb_b = np.asarray(inputs["emb_b"], np.float32)
    xhi = x.astype(ml_dtypes.float8_e4m3)
    xlo = (x - xhi.astype(np.float32)).astype(ml_dtypes.float8_e4m3)
    xa = np.zeros((N, 32), ml_dtypes.float8_e4m3)
    xa[:, 0:9] = xhi
    xa[:, 10:19] = xlo
    xa[:, 20] = np.float32(1.0)
    shared["xg8"] = np.ascontiguousarray(
        xa.reshape(KB2, 2, 128, 32).transpose(2, 0, 1, 3))
    ewa = np.zeros((32, D), np.float32)
    ewa[0:9] = emb_w
    ewa[10:19] = emb_w
    ewa[20] = emb_b
    shared["embwa"] = ewa

    in_maps = []
    ones_row = np.ones((1, NL), np.float32)
    for c in range(NCORES):
        lo = NL * c
        mask = (dst >= lo) & (dst < lo + NL)
        flat = src[mask] * NL + (dst[mask] - lo)
        a = np.bincount(flat, minlength=N * NL).astype(np.float32)
        # paired layout for DoubleRow: [p, kb2, two, dst]
        a = a.reshape(KB2, 2, 128, NL).transpose(2, 0, 1, 3)
        m = dict(shared)
        m["acm8"] = np.ascontiguousarray(a).astype(ml_dtypes.float8_e4m3)
        m["xt"] = np.ascontiguousarray(
            np.vstack([x[lo:lo + NL].T, ones_row]))
        in_maps.append(m)
    return in_maps


def kernel(**inputs) -> np.ndarray:
    global _NC_CACHE
    if _NC_CACHE is None:
        _NC_CACHE = build_nc()
    nc = _NC_CACHE
    in_maps = _host_prep(inputs)
    res = run_bass_kernel_spmd(nc, in_maps, core_ids=list(range(NCORES)))
    out = np.concatenate(
        [np.asarray(res.results[c]["qout"]).reshape(GL, K, NG, NG)
         for c in range(NCORES)], axis=0)
    return out
